# revision 15
# baseline (speedup 1.0000x reference)
"""Trainium2 Bass kernel for nn_Discriminator (GCN + packed MLP), 8 NeuronCores.

Strategy (v3):
  - Kernel 1 (GCN): graphs sharded 8/core. Host does integer-only index
    preprocessing of edge_index (sort edges by dst, pad each node's edge
    list to KMAX slots => CSR grid [128 part, 16 node, KMAX slot]), and
    pure layout gathers of input data: per-slot x[src] (bf16) and per-slot
    / per-node integer degree counts (bf16-encoded ints). The device does
    every float op: dv = rsqrt(degslot), msg = x_slot * dv, u = row-reduce,
    dinv = rsqrt(degnode), out = (u + x_node*dinv)*dinv*w + b.
    No per-edge PE matmuls, no one-hot construction: ~12 instructions per
    graph (ACT rsqrt, DVE mult/reduce, DMA in/out).
  - Kernel 2 (MLP): batch sharded 1250/core, all bf16. The [B,2000] gather
    gcn_out[graph_ids] collapses to a [64,128] table via
    (gcn_out @ gme_w[:2000])[graph_ids] as a one-hot matmul.
"""
import os
import numpy as np

import concourse.bass as bass
import concourse.bacc as bacc
import concourse.mybir as mybir
import concourse.tile as tile
from concourse import bass_utils
from concourse.bass import ds
from concourse.masks import make_identity

P = 128
B, TED, G, N, E, MD, NOISE = 10000, 512, 64, 2000, 64000, 15, 128
PAC = 10
PACDIM = 6400
D0, D1 = 1024, 512
NCORES = 8
GPC = G // NCORES          # graphs per core = 8
BPC = B // NCORES          # batch rows per core = 1250
OPC = BPC // PAC           # output rows per core = 125
NPAD = 2048                # padded node count (128 partitions x 16 nodes)
WN = 16                    # nodes per partition
f32 = mybir.dt.float32
bf16 = mybir.dt.bfloat16
i32 = mybir.dt.int32
AOT = mybir.AluOpType
ACTF = mybir.ActivationFunctionType


def _install_ntff_hook():
    import sys, types
    try:
        from trn_agent_boot.trn_boot import _ntff_profile_via_ctypes
    except Exception:
        return
    if 'antenv.axon_hooks' in sys.modules:
        return
    hook = _ntff_profile_via_ctypes('/opt/axon/libaxon_pjrt.so')
    mod = types.ModuleType('antenv.axon_hooks')
    state = {'hook': hook}
    mod.get_axon_ntff_profile_hook = lambda: state['hook']
    mod.set_axon_ntff_profile_hook = lambda h: state.update(hook=h)
    sys.modules['antenv.axon_hooks'] = mod


# ----------------------------------------------------------------------------
# Kernel 1: GCN over 8 graphs per core (CSR-slot formulation)
# ----------------------------------------------------------------------------

def build_gcn(kmax):
    GB = 2                       # graphs per compute batch
    NB = GPC // GB               # number of batches = 4
    nc = bacc.Bacc("TRN2", target_bir_lowering=False)
    xi_d = nc.dram_tensor("xi", [GPC, P, WN, kmax], bf16, kind="ExternalInput")
    dsl_d = nc.dram_tensor("dsl", [GPC, P, WN, kmax], bf16, kind="ExternalInput")
    dnx_d = nc.dram_tensor("dnx", [P, 2, GPC, WN], f32, kind="ExternalInput")
    wb_d = nc.dram_tensor("wb", [P, 2], f32, kind="ExternalInput")
    go_d = nc.dram_tensor("go", [P, GPC, WN], f32, kind="ExternalOutput")

    with tile.TileContext(nc) as tc:
        with (
            tc.tile_pool(name="const", bufs=1) as cpool,
            tc.tile_pool(name="slotin", bufs=3) as spool,
            tc.tile_pool(name="work", bufs=3) as wpool,
            tc.tile_pool(name="node", bufs=1) as npool,
        ):
            wb = cpool.tile([P, 2], f32)
            nc.gpsimd.dma_start(out=wb[:], in_=wb_d[:])
            dnx = npool.tile([P, 2, GPC, WN], f32)
            nc.gpsimd.dma_start(out=dnx[:], in_=dnx_d[:])
            # dinv[v] = rsqrt(deg[v]+1) for all graphs at once
            dinv = npool.tile([P, GPC, WN], f32)
            nc.scalar.activation(out=dinv[:], in_=dnx[:, 0],
                                 func=ACTF.Abs_reciprocal_sqrt,
                                 bias=0.0, scale=1.0)
            u_all = npool.tile([P, GPC, WN], f32)

            for s in range(NB):
                xi = spool.tile([P, GB, WN, kmax], bf16, tag="xi")
                dsl = spool.tile([P, GB, WN, kmax], bf16, tag="dsl")
                for i in range(GB):
                    nc.sync.dma_start(out=xi[:, i], in_=xi_d[ds(GB * s + i, 1)])
                    nc.scalar.dma_start(out=dsl[:, i],
                                        in_=dsl_d[ds(GB * s + i, 1)])
                # dv = rsqrt(deg[src]+1) per slot; msg = x[src] * dv
                dv = wpool.tile([P, GB, WN, kmax], bf16, tag="dv")
                nc.scalar.activation(out=dv[:], in_=dsl[:],
                                     func=ACTF.Abs_reciprocal_sqrt,
                                     bias=0.0, scale=1.0)
                msg = wpool.tile([P, GB, WN, kmax], bf16, tag="msg")
                nc.vector.tensor_tensor(out=msg[:], in0=xi[:], in1=dv[:],
                                        op=AOT.mult)
                # u[v] = sum_k msg[v, k]
                nc.vector.tensor_reduce(out=u_all[:, GB * s:GB * s + GB, :],
                                        in_=msg[:],
                                        axis=mybir.AxisListType.X, op=AOT.add)

            # go = ((u + x[v]*dinv) * dinv) * w + b   (all graphs batched)
            sf = npool.tile([P, GPC, WN], f32)
            nc.vector.tensor_tensor(out=sf[:], in0=dnx[:, 1], in1=dinv[:],
                                    op=AOT.mult)
            nc.vector.tensor_tensor(out=sf[:], in0=sf[:], in1=u_all[:],
                                    op=AOT.add)
            nc.vector.tensor_tensor(out=sf[:], in0=sf[:], in1=dinv[:],
                                    op=AOT.mult)
            go = npool.tile([P, GPC, WN], f32)
            nc.vector.tensor_scalar(out=go[:], in0=sf[:],
                                    scalar1=wb[:, :1], scalar2=wb[:, 1:2],
                                    op0=AOT.mult, op1=AOT.add)
            nc.gpsimd.dma_start(out=go_d[:], in_=go[:])
    nc.compile()
    return nc


# ----------------------------------------------------------------------------
# Fused single-launch kernel: GCN + P-projection allgather + MLP
# ----------------------------------------------------------------------------

def build_fused(kmax):
    GB = 2                       # graphs per compute batch
    NB = GPC // GB
    nc = bacc.Bacc("TRN2", target_bir_lowering=False, num_devices=NCORES)
    # GCN inputs
    xi_d = nc.dram_tensor("xi", [GPC, P, WN, kmax], bf16, kind="ExternalInput")
    dsl_d = nc.dram_tensor("dsl", [GPC, P, WN, kmax], bf16, kind="ExternalInput")
    dnx_d = nc.dram_tensor("dnx", [P, 2, GPC, WN], f32, kind="ExternalInput")
    wb_d = nc.dram_tensor("wb", [P, 2], f32, kind="ExternalInput")
    gmwv_d = nc.dram_tensor("gmwv", [P, WN, NOISE], bf16, kind="ExternalInput")
    # MLP inputs
    inT_d = nc.dram_tensor("inT", [P, 4, BPC], bf16, kind="ExternalInput")
    amT_d = nc.dram_tensor("amT", [16, BPC], bf16, kind="ExternalInput")
    gid_d = nc.dram_tensor("gid", [BPC], i32, kind="ExternalInput")
    gmewB_d = nc.dram_tensor("gmewB", [32, NOISE], bf16, kind="ExternalInput")
    gmeb_d = nc.dram_tensor("gmeb", [NOISE, 1], f32, kind="ExternalInput")
    metaw_d = nc.dram_tensor("metaw", [16, 32], bf16, kind="ExternalInput")
    metab_d = nc.dram_tensor("metab", [32, 1], f32, kind="ExternalInput")
    w0_d = nc.dram_tensor("w0", [P, 50, D0], bf16, kind="ExternalInput")
    b0_d = nc.dram_tensor("b0", [1, D0], f32, kind="ExternalInput")
    w1_d = nc.dram_tensor("w1", [P, 8, D1], bf16, kind="ExternalInput")
    b1_d = nc.dram_tensor("b1", [1, D1], f32, kind="ExternalInput")
    w2_d = nc.dram_tensor("w2", [P, 4, 1], bf16, kind="ExternalInput")
    b2_d = nc.dram_tensor("b2", [1, 1], f32, kind="ExternalInput")
    y_d = nc.dram_tensor("y", [OPC, 1], f32, kind="ExternalOutput")

    with tile.TileContext(nc) as tc:
        with (
            tc.tile_pool(name="const", bufs=1) as cpool,
            tc.tile_pool(name="slotin", bufs=2) as spool,
            tc.tile_pool(name="gwork", bufs=2) as gwpool,
            tc.tile_pool(name="gnode", bufs=1) as npool,
            tc.tile_pool(name="sbuf", bufs=2) as pool,
            tc.tile_pool(name="w0p", bufs=2) as w0pool,
            tc.tile_pool(name="dram", bufs=1, space="DRAM") as dram,
            tc.tile_pool(name="ps", bufs=1, space="PSUM") as ps,
            tc.tile_pool(name="ps2", bufs=2, space="PSUM") as ps2,
        ):
            # ================= GCN phase (ACT/DVE/DMA; PE idle) ============
            wb = cpool.tile([P, 2], f32)
            nc.gpsimd.dma_start(out=wb[:], in_=wb_d[:])
            dnx = npool.tile([P, 2, GPC, WN], f32)
            nc.gpsimd.dma_start(out=dnx[:], in_=dnx_d[:])
            dinv = npool.tile([P, GPC, WN], f32)
            nc.scalar.activation(out=dinv[:], in_=dnx[:, 0],
                                 func=ACTF.Abs_reciprocal_sqrt,
                                 bias=0.0, scale=1.0)
            u_all = npool.tile([P, GPC, WN], f32)
            for s in range(NB):
                xi = spool.tile([P, GB, WN, kmax], bf16, tag="xi")
                dsl = spool.tile([P, GB, WN, kmax], bf16, tag="dsl")
                for i in range(GB):
                    nc.sync.dma_start(out=xi[:, i], in_=xi_d[ds(GB * s + i, 1)])
                    nc.scalar.dma_start(out=dsl[:, i],
                                        in_=dsl_d[ds(GB * s + i, 1)])
                dv = gwpool.tile([P, GB, WN, kmax], bf16, tag="dv")
                nc.scalar.activation(out=dv[:], in_=dsl[:],
                                     func=ACTF.Abs_reciprocal_sqrt,
                                     bias=0.0, scale=1.0)
                msg = gwpool.tile([P, GB, WN, kmax], bf16, tag="msg")
                nc.vector.tensor_tensor(out=msg[:], in0=xi[:], in1=dv[:],
                                        op=AOT.mult)
                nc.vector.tensor_reduce(out=u_all[:, GB * s:GB * s + GB, :],
                                        in_=msg[:],
                                        axis=mybir.AxisListType.X, op=AOT.add)
            sf = npool.tile([P, GPC, WN], f32)
            nc.vector.tensor_tensor(out=sf[:], in0=dnx[:, 1], in1=dinv[:],
                                    op=AOT.mult)
            nc.vector.tensor_tensor(out=sf[:], in0=sf[:], in1=u_all[:],
                                    op=AOT.add)
            nc.vector.tensor_tensor(out=sf[:], in0=sf[:], in1=dinv[:],
                                    op=AOT.mult)
            gob = npool.tile([P, GPC, WN], bf16)
            nc.vector.tensor_scalar(out=gob[:], in0=sf[:],
                                    scalar1=wb[:, :1], scalar2=wb[:, 1:2],
                                    op0=AOT.mult, op1=AOT.add)

            # ---- P_partial = gcn_out(local graphs) @ gme_w[:2048] ----------
            gmwv = pool.tile([P, WN, NOISE], bf16)
            nc.gpsimd.dma_start(out=gmwv[:], in_=gmwv_d[:])
            Pp_ps = ps.tile([GPC, NOISE], f32, tag="psB")
            for w in range(WN):
                nc.tensor.matmul(out=Pp_ps[:], lhsT=gob[:, :, w],
                                 rhs=gmwv[:, w, :],
                                 start=(w == 0), stop=(w == WN - 1))
            Pp = pool.tile([GPC, NOISE], f32)
            nc.vector.tensor_copy(out=Pp[:], in_=Pp_ps[:])

            # ---- allgather P across the 8 cores ---------------------------
            pin = dram.tile([GPC, NOISE], f32)
            pout = dram.tile([G, NOISE], f32)
            nc.gpsimd.dma_start(out=pin[:], in_=Pp[:])
            nc.gpsimd.collective_compute(
                "AllGather", AOT.bypass,
                replica_groups=[list(range(NCORES))],
                ins=[pin[:].opt()], outs=[pout[:].opt()])
            Pfull = pool.tile([G, NOISE], f32)
            nc.gpsimd.dma_start(out=Pfull[:], in_=pout[:])
            P_sb = pool.tile([G, NOISE], bf16)
            nc.vector.tensor_copy(out=P_sb[:], in_=Pfull[:])

            # ================= MLP phase ===================================
            ident = cpool.tile([P, P], bf16)
            make_identity(nc, ident[:])
            i64c = cpool.tile([G, 1], i32)
            nc.gpsimd.iota(i64c[:], pattern=[[0, 1]], base=0, channel_multiplier=1)
            ones = cpool.tile([1, OPC], bf16)
            nc.vector.memset(ones[:], 1.0)

            gidr = pool.tile([G, BPC], i32)
            nc.scalar.dma_start(out=gidr[:],
                                in_=gid_d[None, :].to_broadcast([G, BPC]))
            ohgid = pool.tile([G, BPC], bf16)
            nc.vector.tensor_tensor(out=ohgid[:], in0=i64c[:].to_broadcast(
                [G, BPC]), in1=gidr[:], op=AOT.is_equal)

            # meta_emb^T = relu(meta_w^T @ all_meta^T + b) -> [32, 1250] bf16
            amT = pool.tile([16, BPC], bf16)
            nc.scalar.dma_start(out=amT[:], in_=amT_d[:])
            mw = pool.tile([16, 32], bf16)
            nc.scalar.dma_start(out=mw[:], in_=metaw_d[:])
            mb = cpool.tile([32, 1], f32)
            nc.scalar.dma_start(out=mb[:], in_=metab_d[:])
            me_ps = ps.tile([32, BPC], f32, tag="psA")
            for c0 in range(0, BPC, 512):
                cw = min(512, BPC - c0)
                nc.tensor.matmul(out=me_ps[:, c0:c0 + cw], lhsT=mw[:],
                                 rhs=amT[:, c0:c0 + cw], start=True, stop=True)
            meT = pool.tile([32, BPC], bf16)
            nc.scalar.activation(out=meT[:], in_=me_ps[:],
                                 func=ACTF.Relu, bias=mb[:, :1], scale=1.0)

            # noise^T = P^T gathered + wB^T @ meta_emb^T + gme_b -> [128, 1250]
            wB = pool.tile([32, NOISE], bf16)
            nc.scalar.dma_start(out=wB[:], in_=gmewB_d[:])
            gb = cpool.tile([NOISE, 1], f32)
            nc.scalar.dma_start(out=gb[:], in_=gmeb_d[:])
            nz_ps = ps.tile([NOISE, BPC], f32, tag="psA")
            for c0 in range(0, BPC, 512):
                cw = min(512, BPC - c0)
                nc.tensor.matmul(out=nz_ps[:, c0:c0 + cw], lhsT=P_sb[:],
                                 rhs=ohgid[:, c0:c0 + cw], start=True, stop=False)
                nc.tensor.matmul(out=nz_ps[:, c0:c0 + cw], lhsT=wB[:],
                                 rhs=meT[:, c0:c0 + cw], start=False, stop=True)
            nzT = pool.tile([NOISE, BPC], bf16)
            nc.scalar.activation(out=nzT[:], in_=nz_ps[:],
                                 func=ACTF.Identity, bias=gb[:, :1], scale=1.0)

            # input^T
            inT = pool.tile([P, 4, BPC], bf16)
            nc.gpsimd.dma_start(out=inT[:], in_=inT_d[:])

            # h1 = lrelu(h @ w0 + b0): psum [125, 1024]
            b0 = pool.tile([1, D0], f32)
            nc.scalar.dma_start(out=b0[:], in_=b0_d[:])
            h1a_ps = ps.tile([OPC, 512], f32, tag="psA")
            h1b_ps = ps.tile([OPC, 512], f32, tag="psA2")
            b0b = pool.tile([1, D0], bf16)
            nc.vector.tensor_copy(out=b0b[:], in_=b0[:])
            order = [c for c in range(50) if c % 5 != 4] + \
                    [c for c in range(50) if c % 5 == 4]
            first = True
            for cg in range(10):
                w0g = w0pool.tile([P, 5, D0], bf16, tag="w0g")
                nc.sync.dma_start(out=w0g[:],
                                  in_=w0_d[:, 5 * cg:5 * cg + 5, :])
                for ci in range(5):
                    c = order[5 * cg + ci]
                    p_, fb = c // 5, c % 5
                    if fb < 4:
                        lhs = inT[:, fb, p_::PAC]
                    else:
                        lhs = nzT[:, p_::PAC]
                    nc.tensor.matmul(out=h1a_ps[:], lhsT=lhs,
                                     rhs=w0g[:, ci, :512],
                                     start=first, stop=False)
                    nc.tensor.matmul(out=h1b_ps[:], lhsT=lhs,
                                     rhs=w0g[:, ci, 512:],
                                     start=first, stop=False)
                    first = False
            nc.tensor.matmul(out=h1a_ps[:], lhsT=ones[:],
                             rhs=b0b[:, :512], start=False, stop=True)
            nc.tensor.matmul(out=h1b_ps[:], lhsT=ones[:],
                             rhs=b0b[:, 512:], start=False, stop=True)
            h1 = pool.tile([OPC, D0], f32)
            h1b = pool.tile([OPC, D0], bf16)
            nc.vector.tensor_scalar(out=h1[:, :512], in0=h1a_ps[:], scalar1=0.2,
                                    scalar2=None, op0=AOT.mult)
            nc.vector.tensor_scalar(out=h1[:, 512:], in0=h1b_ps[:], scalar1=0.2,
                                    scalar2=None, op0=AOT.mult)
            nc.vector.tensor_tensor(out=h1b[:, :512], in0=h1a_ps[:],
                                    in1=h1[:, :512], op=AOT.max)
            nc.vector.tensor_tensor(out=h1b[:, 512:], in0=h1b_ps[:],
                                    in1=h1[:, 512:], op=AOT.max)

            # transpose h1 -> [128, 8, 125]
            h1T = pool.tile([P, 8, OPC], bf16)
            for b in range(8):
                tp = ps2.tile([P, OPC], bf16, tag="tp")
                nc.tensor.transpose(out=tp[:], in_=h1b[:, 128 * b:128 * b + 128],
                                    identity=ident[:OPC, :OPC])
                nc.vector.tensor_copy(out=h1T[:, b, :], in_=tp[:])

            # h2 = lrelu(h1 @ w1 + b1): [125, 512]
            w1 = pool.tile([P, 8, D1], bf16)
            nc.sync.dma_start(out=w1[:], in_=w1_d[:])
            b1 = pool.tile([1, D1], f32)
            nc.scalar.dma_start(out=b1[:], in_=b1_d[:])
            b1b = pool.tile([1, D1], bf16)
            nc.vector.tensor_copy(out=b1b[:], in_=b1[:])
            h2_ps = ps.tile([OPC, D1], f32, tag="psB")
            for c in range(8):
                nc.tensor.matmul(out=h2_ps[:], lhsT=h1T[:, c, :], rhs=w1[:, c, :],
                                 start=(c == 0), stop=False)
            nc.tensor.matmul(out=h2_ps[:], lhsT=ones[:], rhs=b1b[:],
                             start=False, stop=True)
            h2 = pool.tile([OPC, D1], f32)
            nc.vector.tensor_scalar(out=h2[:], in0=h2_ps[:], scalar1=0.2,
                                    scalar2=None, op0=AOT.mult)
            h2b = pool.tile([OPC, D1], bf16)
            nc.vector.tensor_tensor(out=h2b[:], in0=h2_ps[:], in1=h2[:],
                                    op=AOT.max)

            # transpose h2 -> [128, 4, 125]
            h2T = pool.tile([P, 4, OPC], bf16)
            for b in range(4):
                tp2 = ps2.tile([P, OPC], bf16, tag="tp")
                nc.tensor.transpose(out=tp2[:], in_=h2b[:, 128 * b:128 * b + 128],
                                    identity=ident[:OPC, :OPC])
                nc.vector.tensor_copy(out=h2T[:, b, :], in_=tp2[:])

            # y = h2 @ w2 + b2: [125, 1]
            w2 = pool.tile([P, 4, 1], bf16)
            nc.scalar.dma_start(out=w2[:], in_=w2_d[:])
            b2 = pool.tile([1, 1], f32)
            nc.scalar.dma_start(out=b2[:], in_=b2_d[:])
            b2b = pool.tile([1, 1], bf16)
            nc.vector.tensor_copy(out=b2b[:], in_=b2[:])
            y_ps = ps.tile([OPC, 1], f32, tag="psB")
            for c in range(4):
                nc.tensor.matmul(out=y_ps[:], lhsT=h2T[:, c, :], rhs=w2[:, c, :],
                                 start=(c == 0), stop=False)
            nc.tensor.matmul(out=y_ps[:], lhsT=ones[:], rhs=b2b[:],
                             start=False, stop=True)
            y = pool.tile([OPC, 1], f32)
            nc.vector.tensor_copy(out=y[:], in_=y_ps[:])
            nc.sync.dma_start(out=y_d[:], in_=y[:])
    nc.compile()
    return nc


# ----------------------------------------------------------------------------
# Kernel 2: MLP over 1250 batch rows per core (bf16)
# ----------------------------------------------------------------------------

def build_mlp():
    nc = bacc.Bacc("TRN2", target_bir_lowering=False)
    inT_d = nc.dram_tensor("inT", [P, 4, BPC], bf16, kind="ExternalInput")
    amT_d = nc.dram_tensor("amT", [16, BPC], bf16, kind="ExternalInput")
    gid_d = nc.dram_tensor("gid", [BPC], i32, kind="ExternalInput")
    gcnT_d = nc.dram_tensor("gcnT", [P, 16, G], bf16, kind="ExternalInput")
    gmew_d = nc.dram_tensor("gmew", [P, 16, NOISE], bf16, kind="ExternalInput")
    gmewB_d = nc.dram_tensor("gmewB", [32, NOISE], bf16, kind="ExternalInput")
    gmeb_d = nc.dram_tensor("gmeb", [NOISE, 1], f32, kind="ExternalInput")
    metaw_d = nc.dram_tensor("metaw", [16, 32], bf16, kind="ExternalInput")
    metab_d = nc.dram_tensor("metab", [32, 1], f32, kind="ExternalInput")
    w0_d = nc.dram_tensor("w0", [P, 50, D0], bf16, kind="ExternalInput")
    b0_d = nc.dram_tensor("b0", [1, D0], f32, kind="ExternalInput")
    w1_d = nc.dram_tensor("w1", [P, 8, D1], bf16, kind="ExternalInput")
    b1_d = nc.dram_tensor("b1", [1, D1], f32, kind="ExternalInput")
    w2_d = nc.dram_tensor("w2", [P, 4, 1], bf16, kind="ExternalInput")
    b2_d = nc.dram_tensor("b2", [1, 1], f32, kind="ExternalInput")
    y_d = nc.dram_tensor("y", [OPC, 1], f32, kind="ExternalOutput")

    with tile.TileContext(nc) as tc:
        with (
            tc.tile_pool(name="const", bufs=1) as cpool,
            tc.tile_pool(name="sbuf", bufs=2) as pool,
            tc.tile_pool(name="w0p", bufs=2) as w0pool,
            tc.tile_pool(name="ps", bufs=1, space="PSUM") as ps,
            tc.tile_pool(name="ps2", bufs=2, space="PSUM") as ps2,
        ):
            ident = cpool.tile([P, P], bf16)
            make_identity(nc, ident[:])
            i64c = cpool.tile([G, 1], i32)
            nc.gpsimd.iota(i64c[:], pattern=[[0, 1]], base=0, channel_multiplier=1)
            ones = cpool.tile([1, OPC], bf16)
            nc.vector.memset(ones[:], 1.0)

            # one-hot of graph ids (transposed): [64, 1250]
            gidr = pool.tile([G, BPC], i32)
            nc.sync.dma_start(out=gidr[:],
                              in_=gid_d[None, :].to_broadcast([G, BPC]))
            ohgid = pool.tile([G, BPC], bf16)
            nc.vector.tensor_tensor(out=ohgid[:], in0=i64c[:].to_broadcast(
                [G, BPC]), in1=gidr[:], op=AOT.is_equal)

            # P = gcn_out @ gme_w[:2000]  -> [64, 128]
            gcnT = pool.tile([P, 16, G], bf16)
            nc.sync.dma_start(out=gcnT[:], in_=gcnT_d[:])
            gmw = pool.tile([P, 16, NOISE], bf16)
            nc.sync.dma_start(out=gmw[:], in_=gmew_d[:])
            P_ps = ps.tile([G, NOISE], f32, tag="psB")
            for c in range(16):
                nc.tensor.matmul(out=P_ps[:], lhsT=gcnT[:, c, :], rhs=gmw[:, c, :],
                                 start=(c == 0), stop=(c == 15))
            P_sb = pool.tile([G, NOISE], bf16)
            nc.vector.tensor_copy(out=P_sb[:], in_=P_ps[:])

            # meta_emb^T = relu(meta_w^T @ all_meta^T + b) -> [32, 1250] bf16
            amT = pool.tile([16, BPC], bf16)
            nc.sync.dma_start(out=amT[:], in_=amT_d[:])
            mw = pool.tile([16, 32], bf16)
            nc.sync.dma_start(out=mw[:], in_=metaw_d[:])
            mb = cpool.tile([32, 1], f32)
            nc.sync.dma_start(out=mb[:], in_=metab_d[:])
            me_ps = ps.tile([32, BPC], f32, tag="psA")
            for c0 in range(0, BPC, 512):
                cw = min(512, BPC - c0)
                nc.tensor.matmul(out=me_ps[:, c0:c0 + cw], lhsT=mw[:],
                                 rhs=amT[:, c0:c0 + cw], start=True, stop=True)
            meT = pool.tile([32, BPC], bf16)
            nc.scalar.activation(out=meT[:], in_=me_ps[:],
                                 func=ACTF.Relu, bias=mb[:, :1], scale=1.0)

            # noise^T = P^T gathered + wB^T @ meta_emb^T + gme_b -> [128, 1250]
            wB = pool.tile([32, NOISE], bf16)
            nc.sync.dma_start(out=wB[:], in_=gmewB_d[:])
            gb = cpool.tile([NOISE, 1], f32)
            nc.sync.dma_start(out=gb[:], in_=gmeb_d[:])
            nz_ps = ps.tile([NOISE, BPC], f32, tag="psA")
            for c0 in range(0, BPC, 512):
                cw = min(512, BPC - c0)
                nc.tensor.matmul(out=nz_ps[:, c0:c0 + cw], lhsT=P_sb[:],
                                 rhs=ohgid[:, c0:c0 + cw], start=True, stop=False)
                nc.tensor.matmul(out=nz_ps[:, c0:c0 + cw], lhsT=wB[:],
                                 rhs=meT[:, c0:c0 + cw], start=False, stop=True)
            nzT = pool.tile([NOISE, BPC], bf16)
            nc.scalar.activation(out=nzT[:], in_=nz_ps[:],
                                 func=ACTF.Identity, bias=gb[:, :1], scale=1.0)

            # input^T
            inT = pool.tile([P, 4, BPC], bf16)
            nc.sync.dma_start(out=inT[:], in_=inT_d[:])

            # h1 = lrelu(h @ w0 + b0): psum [125, 1024]
            b0 = pool.tile([1, D0], f32)
            nc.sync.dma_start(out=b0[:], in_=b0_d[:])
            h1a_ps = ps.tile([OPC, 512], f32, tag="psA")
            h1b_ps = ps.tile([OPC, 512], f32, tag="psA2")
            b0b = pool.tile([1, D0], bf16)
            nc.vector.tensor_copy(out=b0b[:], in_=b0[:])
            # w0 chunks reordered (host side): input-only chunks (fb<4) first,
            # noise-dependent chunks (fb==4) last, so h1 matmuls start as soon
            # as the first weight group lands.
            order = [c for c in range(50) if c % 5 != 4] + \
                    [c for c in range(50) if c % 5 == 4]
            first = True
            for cg in range(5):
                w0g = w0pool.tile([P, 10, D0], bf16, tag="w0g")
                nc.sync.dma_start(out=w0g[:],
                                  in_=w0_d[:, 10 * cg:10 * cg + 10, :])
                for ci in range(10):
                    c = order[10 * cg + ci]
                    p_, fb = c // 5, c % 5
                    if fb < 4:
                        lhs = inT[:, fb, p_::PAC]
                    else:
                        lhs = nzT[:, p_::PAC]
                    nc.tensor.matmul(out=h1a_ps[:], lhsT=lhs,
                                     rhs=w0g[:, ci, :512],
                                     start=first, stop=False)
                    nc.tensor.matmul(out=h1b_ps[:], lhsT=lhs,
                                     rhs=w0g[:, ci, 512:],
                                     start=first, stop=False)
                    first = False
            nc.tensor.matmul(out=h1a_ps[:], lhsT=ones[:],
                             rhs=b0b[:, :512], start=False, stop=True)
            nc.tensor.matmul(out=h1b_ps[:], lhsT=ones[:],
                             rhs=b0b[:, 512:], start=False, stop=True)
            h1 = pool.tile([OPC, D0], f32)
            h1b = pool.tile([OPC, D0], bf16)
            nc.vector.tensor_scalar(out=h1[:, :512], in0=h1a_ps[:], scalar1=0.2,
                                    scalar2=None, op0=AOT.mult)
            nc.vector.tensor_scalar(out=h1[:, 512:], in0=h1b_ps[:], scalar1=0.2,
                                    scalar2=None, op0=AOT.mult)
            nc.vector.tensor_tensor(out=h1b[:, :512], in0=h1a_ps[:],
                                    in1=h1[:, :512], op=AOT.max)
            nc.vector.tensor_tensor(out=h1b[:, 512:], in0=h1b_ps[:],
                                    in1=h1[:, 512:], op=AOT.max)

            # transpose h1 -> [128, 8, 125]
            h1T = pool.tile([P, 8, OPC], bf16)
            for b in range(8):
                tp = ps2.tile([P, OPC], bf16, tag="tp")
                nc.tensor.transpose(out=tp[:], in_=h1b[:, 128 * b:128 * b + 128],
                                    identity=ident[:OPC, :OPC])
                nc.vector.tensor_copy(out=h1T[:, b, :], in_=tp[:])

            # h2 = lrelu(h1 @ w1 + b1): [125, 512]
            w1 = pool.tile([P, 8, D1], bf16)
            nc.sync.dma_start(out=w1[:], in_=w1_d[:])
            b1 = pool.tile([1, D1], f32)
            nc.sync.dma_start(out=b1[:], in_=b1_d[:])
            b1b = pool.tile([1, D1], bf16)
            nc.vector.tensor_copy(out=b1b[:], in_=b1[:])
            h2_ps = ps.tile([OPC, D1], f32, tag="psB")
            for c in range(8):
                nc.tensor.matmul(out=h2_ps[:], lhsT=h1T[:, c, :], rhs=w1[:, c, :],
                                 start=(c == 0), stop=False)
            nc.tensor.matmul(out=h2_ps[:], lhsT=ones[:], rhs=b1b[:],
                             start=False, stop=True)
            h2 = pool.tile([OPC, D1], f32)
            nc.vector.tensor_scalar(out=h2[:], in0=h2_ps[:], scalar1=0.2,
                                    scalar2=None, op0=AOT.mult)
            h2b = pool.tile([OPC, D1], bf16)
            nc.vector.tensor_tensor(out=h2b[:], in0=h2_ps[:], in1=h2[:],
                                    op=AOT.max)

            # transpose h2 -> [128, 4, 125]
            h2T = pool.tile([P, 4, OPC], bf16)
            for b in range(4):
                tp2 = ps2.tile([P, OPC], bf16, tag="tp")
                nc.tensor.transpose(out=tp2[:], in_=h2b[:, 128 * b:128 * b + 128],
                                    identity=ident[:OPC, :OPC])
                nc.vector.tensor_copy(out=h2T[:, b, :], in_=tp2[:])

            # y = h2 @ w2 + b2: [125, 1]
            w2 = pool.tile([P, 4, 1], bf16)
            nc.sync.dma_start(out=w2[:], in_=w2_d[:])
            b2 = pool.tile([1, 1], f32)
            nc.sync.dma_start(out=b2[:], in_=b2_d[:])
            b2b = pool.tile([1, 1], bf16)
            nc.vector.tensor_copy(out=b2b[:], in_=b2[:])
            y_ps = ps.tile([OPC, 1], f32, tag="psB")
            for c in range(4):
                nc.tensor.matmul(out=y_ps[:], lhsT=h2T[:, c, :], rhs=w2[:, c, :],
                                 start=(c == 0), stop=False)
            nc.tensor.matmul(out=y_ps[:], lhsT=ones[:], rhs=b2b[:],
                             start=False, stop=True)
            y = pool.tile([OPC, 1], f32)
            nc.vector.tensor_copy(out=y[:], in_=y_ps[:])
            nc.sync.dma_start(out=y_d[:], in_=y[:])
    nc.compile()
    return nc


_CACHE = {}
_FUSED = os.environ.get("KFUSED", "1") != "0"


def _programs(kmax):
    if "gcn" not in _CACHE:
        _install_ntff_hook()
        _CACHE["gcn"] = build_gcn(kmax)
        _CACHE["mlp"] = build_mlp()
    return _CACHE["gcn"], _CACHE["mlp"]


def _program_fused(kmax):
    if "fused" not in _CACHE:
        _install_ntff_hook()
        _CACHE["fused"] = build_fused(kmax)
    return _CACHE["fused"]


def _bf(x):
    import ml_dtypes
    return np.asarray(x, dtype=ml_dtypes.bfloat16)


def _csr_prep(graphs_x, edge_index):
    """Integer-only CSR layout prep + pure gathers of input data.

    Returns per-graph slot tensors (x[src] per slot, deg+1 per slot as
    bf16-encoded ints), per-node deg+1 and x. No float arithmetic here;
    everything numeric the device consumes is either gathered input data
    or integer counts of index values.
    """
    f = np.float32
    ei = np.asarray(edge_index, np.int64)
    src, dst = ei[:, 0, :], ei[:, 1, :]            # [G, E]
    xpad = np.zeros((G, NPAD), f)
    xpad[:, :N] = np.asarray(graphs_x, f).reshape(G, N)

    # deg+1 per node (integer bincount of dst indices)
    deg1 = np.ones((G, NPAD), np.int32)
    for g in range(G):
        deg1[g, :N] += np.bincount(dst[g], minlength=N)[:N]
    kmax = int(deg1.max())                          # includes the +1
    kmax = max(16, -(-kmax // 8) * 8)

    # slot position of each edge: k-th edge into its dst node
    order = np.argsort(dst, axis=1, kind='stable')
    dsts = np.take_along_axis(dst, order, axis=1)
    srcs = np.take_along_axis(src, order, axis=1)
    starts = np.zeros((G, NPAD), np.int64)
    for g in range(G):
        cs = np.cumsum(np.bincount(dsts[g], minlength=NPAD))
        starts[g, 1:] = cs[:-1]
    k = np.arange(E)[None, :] - np.take_along_axis(starts, dsts, axis=1)
    flat = dsts * kmax + k                          # [G, E] slot index

    xi = np.zeros((G, NPAD * kmax), f)
    dsl = np.ones((G, NPAD * kmax), np.int32)
    gi = np.arange(G)[:, None]
    xi[gi, flat] = xpad[gi, srcs]
    dsl[gi, flat] = deg1[gi, srcs]
    xi = _bf(xi).reshape(G, P, WN, kmax)
    dsl = _bf(dsl).reshape(G, P, WN, kmax)
    # dnx[core]: [P, {deg+1, x}, GPC, WN] f32
    dn = deg1.astype(f).reshape(NCORES, GPC, P, WN)
    xn = xpad.reshape(NCORES, GPC, P, WN)
    dnx = np.ascontiguousarray(
        np.stack([dn, xn], axis=1).transpose(0, 3, 1, 2, 4))
    return kmax, xi, dsl, dnx


def _mlp_host_prep(input_, graph_ids, chain, metadata, gme_w, gme_b,
                   meta_w, meta_b, seq_w0, seq_b0, seq_w1, seq_b1,
                   seq_w2, seq_b2):
    f = np.float32
    inT_full = np.ascontiguousarray(np.asarray(input_, f).T)
    amT_full = np.ascontiguousarray(
        np.concatenate([np.asarray(chain, f), np.asarray(metadata, f)], axis=1).T)
    w0b = _bf(seq_w0)
    _order = [c for c in range(50) if c % 5 != 4] + \
             [c for c in range(50) if c % 5 == 4]
    w0r = np.ascontiguousarray(
        w0b.reshape(50, P, D0)[_order].transpose(1, 0, 2))
    w1b = np.ascontiguousarray(
        _bf(seq_w1).reshape(8, P, D1).transpose(1, 0, 2))
    w2b = np.ascontiguousarray(
        _bf(seq_w2).reshape(4, P, 1).transpose(1, 0, 2))
    gmewBb = _bf(np.asarray(gme_w, f)[N:, :])
    metawb = _bf(meta_w)
    in2 = []
    for c in range(NCORES):
        sl = slice(BPC * c, BPC * c + BPC)
        in2.append({
            "inT": np.ascontiguousarray(
                _bf(inT_full[:, sl]).reshape(4, P, BPC).transpose(1, 0, 2)),
            "amT": _bf(amT_full[:, sl]),
            "gid": np.ascontiguousarray(np.asarray(graph_ids, np.int32)[sl]),
            "gmewB": gmewBb,
            "gmeb": np.asarray(gme_b, f).reshape(NOISE, 1),
            "metaw": metawb,
            "metab": np.asarray(meta_b, f).reshape(32, 1),
            "w0": w0r,
            "b0": np.asarray(seq_b0, f).reshape(1, D0),
            "w1": w1b, "b1": np.asarray(seq_b1, f).reshape(1, D1),
            "w2": w2b, "b2": np.asarray(seq_b2, f).reshape(1, 1),
        })
    return in2


def kernel(input_, graphs_x, edge_index, graph_ids, chain, metadata,
           gcn_w, gcn_b, meta_w, meta_b, gme_w, gme_b,
           seq_w0, seq_b0, seq_w1, seq_b1, seq_w2, seq_b2,
           _trace=False):
    f = np.float32
    kmax, xi, dsl, dnx = _csr_prep(graphs_x, edge_index)
    wb = np.empty((P, 2), f)
    wb[:, 0] = np.float32(gcn_w.reshape(-1)[0])
    wb[:, 1] = np.float32(gcn_b.reshape(-1)[0])
    gmew_pad = np.zeros((NPAD, NOISE), f)
    gmew_pad[:N, :] = np.asarray(gme_w, f)[:N, :]
    in2 = _mlp_host_prep(input_, graph_ids, chain, metadata, gme_w, gme_b,
                         meta_w, meta_b, seq_w0, seq_b0, seq_w1, seq_b1,
                         seq_w2, seq_b2)

    if _FUSED:
        ncf = _program_fused(kmax)
        gmwvb = np.ascontiguousarray(_bf(gmew_pad).reshape(P, WN, NOISE))
        inf = []
        for c in range(NCORES):
            sl = slice(GPC * c, GPC * c + GPC)
            m = dict(in2[c])
            m.update({
                "xi": np.ascontiguousarray(xi[sl]),
                "dsl": np.ascontiguousarray(dsl[sl]),
                "dnx": dnx[c],
                "wb": wb,
                "gmwv": gmwvb,
            })
            inf.append(m)
        r = bass_utils.run_bass_kernel_spmd(ncf, inf,
                                            core_ids=list(range(NCORES)),
                                            trace=_trace)
        y = np.concatenate([np.asarray(r.results[c]["y"], f)
                            for c in range(NCORES)], axis=0)
        kernel.last_exec_ns = ((r.exec_time_ns or 0), 0)
        return y

    nc1, nc2 = _programs(kmax)
    in1 = []
    for c in range(NCORES):
        sl = slice(GPC * c, GPC * c + GPC)
        in1.append({
            "xi": np.ascontiguousarray(xi[sl]),
            "dsl": np.ascontiguousarray(dsl[sl]),
            "dnx": dnx[c],
            "wb": wb,
        })
    r1 = bass_utils.run_bass_kernel_spmd(nc1, in1, core_ids=list(range(NCORES)),
                                         trace=_trace)
    # go: [P, GPC, WN] f32 per core, node v = 16*p + w -> gcnT [NPAD, G]
    gcnT = np.concatenate(
        [np.asarray(r1.results[c]["go"], f).transpose(1, 0, 2).reshape(GPC, NPAD)
         for c in range(NCORES)], axis=0).T

    gcnTb = np.ascontiguousarray(
        _bf(gcnT).reshape(16, P, G).transpose(1, 0, 2))
    gmewb = np.ascontiguousarray(
        _bf(gmew_pad).reshape(16, P, NOISE).transpose(1, 0, 2))
    for c in range(NCORES):
        in2[c]["gcnT"] = gcnTb
        in2[c]["gmew"] = gmewb
    r2 = bass_utils.run_bass_kernel_spmd(nc2, in2, core_ids=list(range(NCORES)),
                                         trace=_trace)
    y = np.concatenate([np.asarray(r2.results[c]["y"], f)
                        for c in range(NCORES)], axis=0)
    kernel.last_exec_ns = ((r1.exec_time_ns or 0), (r2.exec_time_ns or 0))
    return y


# revision 18
# speedup vs baseline: 1.4217x; 1.4217x over previous
"""Trainium2 Bass kernel for nn_Discriminator (GCN + packed MLP), 8 NeuronCores.

Strategy (v3):
  - Kernel 1 (GCN): graphs sharded 8/core. Host does integer-only index
    preprocessing of edge_index (sort edges by dst, pad each node's edge
    list to KMAX slots => CSR grid [128 part, 16 node, KMAX slot]), and
    pure layout gathers of input data: per-slot x[src] (bf16) and per-slot
    / per-node integer degree counts (bf16-encoded ints). The device does
    every float op: dv = rsqrt(degslot), msg = x_slot * dv, u = row-reduce,
    dinv = rsqrt(degnode), out = (u + x_node*dinv)*dinv*w + b.
    No per-edge PE matmuls, no one-hot construction: ~12 instructions per
    graph (ACT rsqrt, DVE mult/reduce, DMA in/out).
  - Kernel 2 (MLP): batch sharded 1250/core, all bf16. The [B,2000] gather
    gcn_out[graph_ids] collapses to a [64,128] table via
    (gcn_out @ gme_w[:2000])[graph_ids] as a one-hot matmul.
"""
import os
import numpy as np

import concourse.bass as bass
import concourse.bacc as bacc
import concourse.mybir as mybir
import concourse.tile as tile
from concourse import bass_utils
from concourse.bass import ds
from concourse.masks import make_identity

P = 128
B, TED, G, N, E, MD, NOISE = 10000, 512, 64, 2000, 64000, 15, 128
PAC = 10
PACDIM = 6400
D0, D1 = 1024, 512
NCORES = 8
GPC = G // NCORES          # graphs per core = 8
BPC = B // NCORES          # batch rows per core = 1250
OPC = BPC // PAC           # output rows per core = 125
NPAD = 2048                # padded node count (128 partitions x 16 nodes)
WN = 16                    # nodes per partition
f32 = mybir.dt.float32
bf16 = mybir.dt.bfloat16
i32 = mybir.dt.int32
AOT = mybir.AluOpType
ACTF = mybir.ActivationFunctionType


def _install_ntff_hook():
    import sys, types
    try:
        from trn_agent_boot.trn_boot import _ntff_profile_via_ctypes
    except Exception:
        return
    if 'antenv.axon_hooks' in sys.modules:
        return
    hook = _ntff_profile_via_ctypes('/opt/axon/libaxon_pjrt.so')
    mod = types.ModuleType('antenv.axon_hooks')
    state = {'hook': hook}
    mod.get_axon_ntff_profile_hook = lambda: state['hook']
    mod.set_axon_ntff_profile_hook = lambda h: state.update(hook=h)
    sys.modules['antenv.axon_hooks'] = mod


# ----------------------------------------------------------------------------
# Kernel 1: GCN over 8 graphs per core (CSR-slot formulation)
# ----------------------------------------------------------------------------

def build_gcn(kmax):
    GB = 2                       # graphs per compute batch
    NB = GPC // GB               # number of batches = 4
    nc = bacc.Bacc("TRN2", target_bir_lowering=False)
    xi_d = nc.dram_tensor("xi", [GPC, P, WN, kmax], bf16, kind="ExternalInput")
    dsl_d = nc.dram_tensor("dsl", [GPC, P, WN, kmax], bf16, kind="ExternalInput")
    dnx_d = nc.dram_tensor("dnx", [P, 2, GPC, WN], f32, kind="ExternalInput")
    wb_d = nc.dram_tensor("wb", [P, 2], f32, kind="ExternalInput")
    go_d = nc.dram_tensor("go", [P, GPC, WN], f32, kind="ExternalOutput")

    with tile.TileContext(nc) as tc:
        with (
            tc.tile_pool(name="const", bufs=1) as cpool,
            tc.tile_pool(name="slotin", bufs=3) as spool,
            tc.tile_pool(name="work", bufs=3) as wpool,
            tc.tile_pool(name="node", bufs=1) as npool,
        ):
            wb = cpool.tile([P, 2], f32)
            nc.gpsimd.dma_start(out=wb[:], in_=wb_d[:])
            dnx = npool.tile([P, 2, GPC, WN], f32)
            nc.gpsimd.dma_start(out=dnx[:], in_=dnx_d[:])
            # dinv[v] = rsqrt(deg[v]+1) for all graphs at once
            dinv = npool.tile([P, GPC, WN], f32)
            nc.scalar.activation(out=dinv[:], in_=dnx[:, 0],
                                 func=ACTF.Abs_reciprocal_sqrt,
                                 bias=0.0, scale=1.0)
            u_all = npool.tile([P, GPC, WN], f32)

            for s in range(NB):
                xi = spool.tile([P, GB, WN, kmax], bf16, tag="xi")
                dsl = spool.tile([P, GB, WN, kmax], bf16, tag="dsl")
                for i in range(GB):
                    nc.sync.dma_start(out=xi[:, i], in_=xi_d[ds(GB * s + i, 1)])
                    nc.scalar.dma_start(out=dsl[:, i],
                                        in_=dsl_d[ds(GB * s + i, 1)])
                # dv = rsqrt(deg[src]+1) per slot; msg = x[src] * dv
                dv = wpool.tile([P, GB, WN, kmax], bf16, tag="dv")
                nc.scalar.activation(out=dv[:], in_=dsl[:],
                                     func=ACTF.Abs_reciprocal_sqrt,
                                     bias=0.0, scale=1.0)
                msg = wpool.tile([P, GB, WN, kmax], bf16, tag="msg")
                nc.vector.tensor_tensor(out=msg[:], in0=xi[:], in1=dv[:],
                                        op=AOT.mult)
                # u[v] = sum_k msg[v, k]
                nc.vector.tensor_reduce(out=u_all[:, GB * s:GB * s + GB, :],
                                        in_=msg[:],
                                        axis=mybir.AxisListType.X, op=AOT.add)

            # go = ((u + x[v]*dinv) * dinv) * w + b   (all graphs batched)
            sf = npool.tile([P, GPC, WN], f32)
            nc.vector.tensor_tensor(out=sf[:], in0=dnx[:, 1], in1=dinv[:],
                                    op=AOT.mult)
            nc.vector.tensor_tensor(out=sf[:], in0=sf[:], in1=u_all[:],
                                    op=AOT.add)
            nc.vector.tensor_tensor(out=sf[:], in0=sf[:], in1=dinv[:],
                                    op=AOT.mult)
            go = npool.tile([P, GPC, WN], f32)
            nc.vector.tensor_scalar(out=go[:], in0=sf[:],
                                    scalar1=wb[:, :1], scalar2=wb[:, 1:2],
                                    op0=AOT.mult, op1=AOT.add)
            nc.gpsimd.dma_start(out=go_d[:], in_=go[:])
    nc.compile()
    return nc


# ----------------------------------------------------------------------------
# Fused single-launch kernel: GCN + P-projection allgather + MLP
# ----------------------------------------------------------------------------

def build_fused(kmax):
    GB = 2                       # graphs per compute batch
    NB = GPC // GB
    nc = bacc.Bacc("TRN2", target_bir_lowering=False, num_devices=NCORES)
    # GCN inputs
    xi_d = nc.dram_tensor("xi", [GPC, P, WN, kmax], bf16, kind="ExternalInput")
    dsl_d = nc.dram_tensor("dsl", [GPC, P, WN, kmax], bf16, kind="ExternalInput")
    dnx_d = nc.dram_tensor("dnx", [P, 2, GPC, WN], f32, kind="ExternalInput")
    wb_d = nc.dram_tensor("wb", [P, 2], f32, kind="ExternalInput")
    gmwv_d = nc.dram_tensor("gmwv", [P, WN, NOISE], bf16, kind="ExternalInput")
    # MLP inputs
    inT_d = nc.dram_tensor("inT", [P, 4, BPC], bf16, kind="ExternalInput")
    amT_d = nc.dram_tensor("amT", [16, BPC], bf16, kind="ExternalInput")
    gid_d = nc.dram_tensor("gid", [BPC], i32, kind="ExternalInput")
    gmewB_d = nc.dram_tensor("gmewB", [32, NOISE], bf16, kind="ExternalInput")
    gmeb_d = nc.dram_tensor("gmeb", [NOISE, 1], f32, kind="ExternalInput")
    metaw_d = nc.dram_tensor("metaw", [16, 32], bf16, kind="ExternalInput")
    metab_d = nc.dram_tensor("metab", [32, 1], f32, kind="ExternalInput")
    w0_d = nc.dram_tensor("w0", [P, 50, D0], bf16, kind="ExternalInput")
    b0_d = nc.dram_tensor("b0", [1, D0], f32, kind="ExternalInput")
    w1_d = nc.dram_tensor("w1", [P, 8, D1], bf16, kind="ExternalInput")
    b1_d = nc.dram_tensor("b1", [1, D1], f32, kind="ExternalInput")
    w2_d = nc.dram_tensor("w2", [P, 4, 1], bf16, kind="ExternalInput")
    b2_d = nc.dram_tensor("b2", [1, 1], f32, kind="ExternalInput")
    y_d = nc.dram_tensor("y", [OPC, 1], f32, kind="ExternalOutput")

    with tile.TileContext(nc) as tc:
        with (
            tc.tile_pool(name="const", bufs=1) as cpool,
            tc.tile_pool(name="slotin", bufs=2) as spool,
            tc.tile_pool(name="gwork", bufs=2) as gwpool,
            tc.tile_pool(name="gnode", bufs=1) as npool,
            tc.tile_pool(name="sbuf", bufs=2) as pool,
            tc.tile_pool(name="w0p", bufs=2) as w0pool,
            tc.tile_pool(name="dram", bufs=1, space="DRAM") as dram,
            tc.tile_pool(name="ps", bufs=1, space="PSUM") as ps,
            tc.tile_pool(name="ps2", bufs=2, space="PSUM") as ps2,
        ):
            # ================= GCN phase (ACT/DVE/DMA; PE idle) ============
            wb = cpool.tile([P, 2], f32)
            nc.gpsimd.dma_start(out=wb[:], in_=wb_d[:])
            dnx = npool.tile([P, 2, GPC, WN], f32)
            nc.gpsimd.dma_start(out=dnx[:], in_=dnx_d[:])
            dinv = npool.tile([P, GPC, WN], f32)
            nc.scalar.activation(out=dinv[:], in_=dnx[:, 0],
                                 func=ACTF.Abs_reciprocal_sqrt,
                                 bias=0.0, scale=1.0)
            u_all = npool.tile([P, GPC, WN], f32)
            for s in range(NB):
                xi = spool.tile([P, GB, WN, kmax], bf16, tag="xi")
                dsl = spool.tile([P, GB, WN, kmax], bf16, tag="dsl")
                for i in range(GB):
                    nc.sync.dma_start(out=xi[:, i], in_=xi_d[ds(GB * s + i, 1)])
                    nc.scalar.dma_start(out=dsl[:, i],
                                        in_=dsl_d[ds(GB * s + i, 1)])
                dv = gwpool.tile([P, GB, WN, kmax], bf16, tag="dv")
                nc.scalar.activation(out=dv[:], in_=dsl[:],
                                     func=ACTF.Abs_reciprocal_sqrt,
                                     bias=0.0, scale=1.0)
                msg = gwpool.tile([P, GB, WN, kmax], bf16, tag="msg")
                nc.vector.tensor_tensor(out=msg[:], in0=xi[:], in1=dv[:],
                                        op=AOT.mult)
                nc.vector.tensor_reduce(out=u_all[:, GB * s:GB * s + GB, :],
                                        in_=msg[:],
                                        axis=mybir.AxisListType.X, op=AOT.add)
            sf = npool.tile([P, GPC, WN], f32)
            nc.vector.tensor_tensor(out=sf[:], in0=dnx[:, 1], in1=dinv[:],
                                    op=AOT.mult)
            nc.vector.tensor_tensor(out=sf[:], in0=sf[:], in1=u_all[:],
                                    op=AOT.add)
            nc.vector.tensor_tensor(out=sf[:], in0=sf[:], in1=dinv[:],
                                    op=AOT.mult)
            gob = npool.tile([P, GPC, WN], bf16)
            nc.vector.tensor_scalar(out=gob[:], in0=sf[:],
                                    scalar1=wb[:, :1], scalar2=wb[:, 1:2],
                                    op0=AOT.mult, op1=AOT.add)

            # ---- P_partial = gcn_out(local graphs) @ gme_w[:2048] ----------
            gmwv = pool.tile([P, WN, NOISE], bf16)
            nc.gpsimd.dma_start(out=gmwv[:], in_=gmwv_d[:])
            Pp_ps = ps.tile([GPC, NOISE], f32, tag="psB")
            for w in range(WN):
                nc.tensor.matmul(out=Pp_ps[:], lhsT=gob[:, :, w],
                                 rhs=gmwv[:, w, :],
                                 start=(w == 0), stop=(w == WN - 1))
            Pp = pool.tile([GPC, NOISE], f32)
            nc.vector.tensor_copy(out=Pp[:], in_=Pp_ps[:])

            # ---- allgather P across the 8 cores ---------------------------
            pin = dram.tile([GPC, NOISE], f32)
            pout = dram.tile([G, NOISE], f32)
            nc.gpsimd.dma_start(out=pin[:], in_=Pp[:])
            nc.gpsimd.collective_compute(
                "AllGather", AOT.bypass,
                replica_groups=[list(range(NCORES))],
                ins=[pin[:].opt()], outs=[pout[:].opt()])
            Pfull = pool.tile([G, NOISE], f32)
            nc.gpsimd.dma_start(out=Pfull[:], in_=pout[:])
            P_sb = pool.tile([G, NOISE], bf16)
            nc.vector.tensor_copy(out=P_sb[:], in_=Pfull[:])

            # ================= MLP phase ===================================
            ident = cpool.tile([P, P], bf16)
            make_identity(nc, ident[:])
            i64c = cpool.tile([G, 1], i32)
            nc.gpsimd.iota(i64c[:], pattern=[[0, 1]], base=0, channel_multiplier=1)
            ones = cpool.tile([1, OPC], bf16)
            nc.vector.memset(ones[:], 1.0)

            gidr = pool.tile([G, BPC], i32)
            nc.scalar.dma_start(out=gidr[:],
                                in_=gid_d[None, :].to_broadcast([G, BPC]))
            ohgid = pool.tile([G, BPC], bf16)
            nc.vector.tensor_tensor(out=ohgid[:], in0=i64c[:].to_broadcast(
                [G, BPC]), in1=gidr[:], op=AOT.is_equal)

            # meta_emb^T = relu(meta_w^T @ all_meta^T + b) -> [32, 1250] bf16
            amT = pool.tile([16, BPC], bf16)
            nc.scalar.dma_start(out=amT[:], in_=amT_d[:])
            mw = pool.tile([16, 32], bf16)
            nc.scalar.dma_start(out=mw[:], in_=metaw_d[:])
            mb = cpool.tile([32, 1], f32)
            nc.scalar.dma_start(out=mb[:], in_=metab_d[:])
            me_ps = ps.tile([32, BPC], f32, tag="psA")
            for c0 in range(0, BPC, 512):
                cw = min(512, BPC - c0)
                nc.tensor.matmul(out=me_ps[:, c0:c0 + cw], lhsT=mw[:],
                                 rhs=amT[:, c0:c0 + cw], start=True, stop=True)
            meT = pool.tile([32, BPC], bf16)
            nc.scalar.activation(out=meT[:], in_=me_ps[:],
                                 func=ACTF.Relu, bias=mb[:, :1], scale=1.0)

            # noise^T = P^T gathered + wB^T @ meta_emb^T + gme_b -> [128, 1250]
            wB = pool.tile([32, NOISE], bf16)
            nc.scalar.dma_start(out=wB[:], in_=gmewB_d[:])
            gb = cpool.tile([NOISE, 1], f32)
            nc.scalar.dma_start(out=gb[:], in_=gmeb_d[:])
            nz_ps = ps.tile([NOISE, BPC], f32, tag="psA")
            for c0 in range(0, BPC, 512):
                cw = min(512, BPC - c0)
                nc.tensor.matmul(out=nz_ps[:, c0:c0 + cw], lhsT=P_sb[:],
                                 rhs=ohgid[:, c0:c0 + cw], start=True, stop=False)
                nc.tensor.matmul(out=nz_ps[:, c0:c0 + cw], lhsT=wB[:],
                                 rhs=meT[:, c0:c0 + cw], start=False, stop=True)
            nzT = pool.tile([NOISE, BPC], bf16)
            nc.scalar.activation(out=nzT[:], in_=nz_ps[:],
                                 func=ACTF.Identity, bias=gb[:, :1], scale=1.0)

            # input^T
            inT = pool.tile([P, 4, BPC], bf16)
            nc.gpsimd.dma_start(out=inT[:], in_=inT_d[:])

            # h1 = lrelu(h @ w0 + b0): psum [125, 1024]
            b0 = pool.tile([1, D0], f32)
            nc.scalar.dma_start(out=b0[:], in_=b0_d[:])
            h1a_ps = ps.tile([OPC, 512], f32, tag="psA")
            h1b_ps = ps.tile([OPC, 512], f32, tag="psA2")
            b0b = pool.tile([1, D0], bf16)
            nc.vector.tensor_copy(out=b0b[:], in_=b0[:])
            order = [c for c in range(50) if c % 5 != 4] + \
                    [c for c in range(50) if c % 5 == 4]
            first = True
            for cg in range(10):
                w0g = w0pool.tile([P, 5, D0], bf16, tag="w0g")
                nc.sync.dma_start(out=w0g[:],
                                  in_=w0_d[:, 5 * cg:5 * cg + 5, :])
                for ci in range(5):
                    c = order[5 * cg + ci]
                    p_, fb = c // 5, c % 5
                    if fb < 4:
                        lhs = inT[:, fb, p_::PAC]
                    else:
                        lhs = nzT[:, p_::PAC]
                    nc.tensor.matmul(out=h1a_ps[:], lhsT=lhs,
                                     rhs=w0g[:, ci, :512],
                                     start=first, stop=False)
                    nc.tensor.matmul(out=h1b_ps[:], lhsT=lhs,
                                     rhs=w0g[:, ci, 512:],
                                     start=first, stop=False)
                    first = False
            nc.tensor.matmul(out=h1a_ps[:], lhsT=ones[:],
                             rhs=b0b[:, :512], start=False, stop=True)
            nc.tensor.matmul(out=h1b_ps[:], lhsT=ones[:],
                             rhs=b0b[:, 512:], start=False, stop=True)
            h1 = pool.tile([OPC, D0], f32)
            h1b = pool.tile([OPC, D0], bf16)
            nc.vector.tensor_scalar(out=h1[:, :512], in0=h1a_ps[:], scalar1=0.2,
                                    scalar2=None, op0=AOT.mult)
            nc.vector.tensor_scalar(out=h1[:, 512:], in0=h1b_ps[:], scalar1=0.2,
                                    scalar2=None, op0=AOT.mult)
            nc.vector.tensor_tensor(out=h1b[:, :512], in0=h1a_ps[:],
                                    in1=h1[:, :512], op=AOT.max)
            nc.vector.tensor_tensor(out=h1b[:, 512:], in0=h1b_ps[:],
                                    in1=h1[:, 512:], op=AOT.max)

            # transpose h1 -> [128, 8, 125]
            h1T = pool.tile([P, 8, OPC], bf16)
            for b in range(8):
                tp = ps2.tile([P, OPC], bf16, tag="tp")
                nc.tensor.transpose(out=tp[:], in_=h1b[:, 128 * b:128 * b + 128],
                                    identity=ident[:OPC, :OPC])
                nc.vector.tensor_copy(out=h1T[:, b, :], in_=tp[:])

            # h2 = lrelu(h1 @ w1 + b1): [125, 512]
            w1 = pool.tile([P, 8, D1], bf16)
            nc.sync.dma_start(out=w1[:], in_=w1_d[:])
            b1 = pool.tile([1, D1], f32)
            nc.scalar.dma_start(out=b1[:], in_=b1_d[:])
            b1b = pool.tile([1, D1], bf16)
            nc.vector.tensor_copy(out=b1b[:], in_=b1[:])
            h2_ps = ps.tile([OPC, D1], f32, tag="psB")
            for c in range(8):
                nc.tensor.matmul(out=h2_ps[:], lhsT=h1T[:, c, :], rhs=w1[:, c, :],
                                 start=(c == 0), stop=False)
            nc.tensor.matmul(out=h2_ps[:], lhsT=ones[:], rhs=b1b[:],
                             start=False, stop=True)
            h2 = pool.tile([OPC, D1], f32)
            nc.vector.tensor_scalar(out=h2[:], in0=h2_ps[:], scalar1=0.2,
                                    scalar2=None, op0=AOT.mult)
            h2b = pool.tile([OPC, D1], bf16)
            nc.vector.tensor_tensor(out=h2b[:], in0=h2_ps[:], in1=h2[:],
                                    op=AOT.max)

            # transpose h2 -> [128, 4, 125]
            h2T = pool.tile([P, 4, OPC], bf16)
            for b in range(4):
                tp2 = ps2.tile([P, OPC], bf16, tag="tp")
                nc.tensor.transpose(out=tp2[:], in_=h2b[:, 128 * b:128 * b + 128],
                                    identity=ident[:OPC, :OPC])
                nc.vector.tensor_copy(out=h2T[:, b, :], in_=tp2[:])

            # y = h2 @ w2 + b2: [125, 1]
            w2 = pool.tile([P, 4, 1], bf16)
            nc.scalar.dma_start(out=w2[:], in_=w2_d[:])
            b2 = pool.tile([1, 1], f32)
            nc.scalar.dma_start(out=b2[:], in_=b2_d[:])
            b2b = pool.tile([1, 1], bf16)
            nc.vector.tensor_copy(out=b2b[:], in_=b2[:])
            y_ps = ps.tile([OPC, 1], f32, tag="psB")
            for c in range(4):
                nc.tensor.matmul(out=y_ps[:], lhsT=h2T[:, c, :], rhs=w2[:, c, :],
                                 start=(c == 0), stop=False)
            nc.tensor.matmul(out=y_ps[:], lhsT=ones[:], rhs=b2b[:],
                             start=False, stop=True)
            y = pool.tile([OPC, 1], f32)
            nc.vector.tensor_copy(out=y[:], in_=y_ps[:])
            nc.sync.dma_start(out=y_d[:], in_=y[:])
    nc.compile()
    return nc


# ----------------------------------------------------------------------------
# Kernel 2: MLP over 1250 batch rows per core (bf16)
# ----------------------------------------------------------------------------

# ----------------------------------------------------------------------------
# Split pipeline: L1 = GCN + P-projection + meta + input-only h1 chunks;
# L2 = noise-dependent h1 chunks + h2 + y. No cross-core sync anywhere.
# ----------------------------------------------------------------------------

NIN1 = 40                       # input-only w0 chunks done in launch 1


def build_l1(kmax):
    GB = 2
    NB = GPC // GB
    nc = bacc.Bacc("TRN2", target_bir_lowering=False)
    xi_d = nc.dram_tensor("xi", [GPC, P, WN, kmax], bf16, kind="ExternalInput")
    dsl_d = nc.dram_tensor("dsl", [GPC, P, WN, kmax], bf16, kind="ExternalInput")
    dnx_d = nc.dram_tensor("dnx", [P, 2, GPC, WN], f32, kind="ExternalInput")
    wb_d = nc.dram_tensor("wb", [P, 2], f32, kind="ExternalInput")
    gmwv_d = nc.dram_tensor("gmwv", [P, WN, NOISE], bf16, kind="ExternalInput")
    inT_d = nc.dram_tensor("inT", [P, 4, BPC], bf16, kind="ExternalInput")
    w0a_d = nc.dram_tensor("w0a", [P, NIN1, D0], bf16, kind="ExternalInput")
    amT_d = nc.dram_tensor("amT", [16, BPC], bf16, kind="ExternalInput")
    metaw_d = nc.dram_tensor("metaw", [16, 32], bf16, kind="ExternalInput")
    metab_d = nc.dram_tensor("metab", [32, 1], f32, kind="ExternalInput")
    pp_d = nc.dram_tensor("pp", [GPC, NOISE], f32, kind="ExternalOutput")
    h1p_d = nc.dram_tensor("h1p", [OPC, D0], f32, kind="ExternalOutput")
    meTo_d = nc.dram_tensor("meTo", [32, BPC], bf16, kind="ExternalOutput")

    with tile.TileContext(nc) as tc:
        with (
            tc.tile_pool(name="const", bufs=1) as cpool,
            tc.tile_pool(name="slotin", bufs=2) as spool,
            tc.tile_pool(name="gwork", bufs=2) as gwpool,
            tc.tile_pool(name="gnode", bufs=1) as npool,
            tc.tile_pool(name="sbuf", bufs=1) as pool,
            tc.tile_pool(name="w0p", bufs=2) as w0pool,
            tc.tile_pool(name="ps", bufs=1, space="PSUM") as ps,
        ):
            # ---- MLP-side DMAs issued up front on spare queues ----
            inT = pool.tile([P, 4, BPC], bf16)
            nc.gpsimd.dma_start(out=inT[:], in_=inT_d[:])
            amT = pool.tile([16, BPC], bf16)
            nc.gpsimd.dma_start(out=amT[:], in_=amT_d[:])
            mw = pool.tile([16, 32], bf16)
            nc.gpsimd.dma_start(out=mw[:], in_=metaw_d[:])
            mb = cpool.tile([32, 1], f32)
            nc.gpsimd.dma_start(out=mb[:], in_=metab_d[:])
            gmwv = pool.tile([P, WN, NOISE], bf16)
            nc.gpsimd.dma_start(out=gmwv[:], in_=gmwv_d[:])

            # ---- GCN ----
            wb = cpool.tile([P, 2], f32)
            nc.gpsimd.dma_start(out=wb[:], in_=wb_d[:])
            dnx = npool.tile([P, 2, GPC, WN], f32)
            nc.gpsimd.dma_start(out=dnx[:], in_=dnx_d[:])
            dinv = npool.tile([P, GPC, WN], f32)
            nc.scalar.activation(out=dinv[:], in_=dnx[:, 0],
                                 func=ACTF.Abs_reciprocal_sqrt,
                                 bias=0.0, scale=1.0)
            u_all = npool.tile([P, GPC, WN], f32)
            for s in range(NB):
                xi = spool.tile([P, GB, WN, kmax], bf16, tag="xi")
                dsl = spool.tile([P, GB, WN, kmax], bf16, tag="dsl")
                for i in range(GB):
                    nc.sync.dma_start(out=xi[:, i], in_=xi_d[ds(GB * s + i, 1)])
                    nc.scalar.dma_start(out=dsl[:, i],
                                        in_=dsl_d[ds(GB * s + i, 1)])
                dv = gwpool.tile([P, GB, WN, kmax], bf16, tag="dv")
                nc.scalar.activation(out=dv[:], in_=dsl[:],
                                     func=ACTF.Abs_reciprocal_sqrt,
                                     bias=0.0, scale=1.0)
                msg = gwpool.tile([P, GB, WN, kmax], bf16, tag="msg")
                nc.vector.tensor_tensor(out=msg[:], in0=xi[:], in1=dv[:],
                                        op=AOT.mult)
                nc.vector.tensor_reduce(out=u_all[:, GB * s:GB * s + GB, :],
                                        in_=msg[:],
                                        axis=mybir.AxisListType.X, op=AOT.add)
            sf = npool.tile([P, GPC, WN], f32)
            nc.vector.tensor_tensor(out=sf[:], in0=dnx[:, 1], in1=dinv[:],
                                    op=AOT.mult)
            nc.vector.tensor_tensor(out=sf[:], in0=sf[:], in1=u_all[:],
                                    op=AOT.add)
            nc.vector.tensor_tensor(out=sf[:], in0=sf[:], in1=dinv[:],
                                    op=AOT.mult)
            gob = npool.tile([P, GPC, WN], bf16)
            nc.vector.tensor_scalar(out=gob[:], in0=sf[:],
                                    scalar1=wb[:, :1], scalar2=wb[:, 1:2],
                                    op0=AOT.mult, op1=AOT.add)

            # ---- P_partial = gcn_out(local graphs) @ gme_w ----
            Pp_ps = ps.tile([GPC, NOISE], f32, tag="psP")
            for w in range(WN):
                nc.tensor.matmul(out=Pp_ps[:], lhsT=gob[:, :, w],
                                 rhs=gmwv[:, w, :],
                                 start=(w == 0), stop=(w == WN - 1))
            Pp = npool.tile([GPC, NOISE], f32)
            nc.vector.tensor_copy(out=Pp[:], in_=Pp_ps[:])
            nc.gpsimd.dma_start(out=pp_d[:], in_=Pp[:])

            # ---- meta_emb^T = relu(meta_w^T @ all_meta^T + b) ----
            me_ps = ps.tile([32, BPC], f32, tag="psM")
            for c0 in range(0, BPC, 512):
                cw = min(512, BPC - c0)
                nc.tensor.matmul(out=me_ps[:, c0:c0 + cw], lhsT=mw[:],
                                 rhs=amT[:, c0:c0 + cw], start=True, stop=True)
            meT = pool.tile([32, BPC], bf16)
            nc.scalar.activation(out=meT[:], in_=me_ps[:],
                                 func=ACTF.Relu, bias=mb[:, :1], scale=1.0)
            nc.gpsimd.dma_start(out=meTo_d[:], in_=meT[:])

            # ---- h1 partial: input-only chunks ----
            h1a_ps = ps.tile([OPC, 512], f32, tag="psA")
            h1b_ps = ps.tile([OPC, 512], f32, tag="psA2")
            order = [c for c in range(50) if c % 5 != 4] + \
                    [c for c in range(50) if c % 5 == 4]
            first = True
            for cg in range(NIN1 // 5):
                w0g = w0pool.tile([P, 5, D0], bf16, tag="w0g")
                nc.sync.dma_start(out=w0g[:],
                                  in_=w0a_d[:, 5 * cg:5 * cg + 5, :])
                for ci in range(5):
                    c = order[5 * cg + ci]
                    p_, fb = c // 5, c % 5
                    lhs = inT[:, fb, p_::PAC]
                    nc.tensor.matmul(out=h1a_ps[:], lhsT=lhs,
                                     rhs=w0g[:, ci, :512],
                                     start=first, stop=(cg == NIN1 // 5 - 1
                                                        and ci == 4))
                    nc.tensor.matmul(out=h1b_ps[:], lhsT=lhs,
                                     rhs=w0g[:, ci, 512:],
                                     start=first, stop=(cg == NIN1 // 5 - 1
                                                        and ci == 4))
                    first = False
            h1p = pool.tile([OPC, D0], f32)
            nc.vector.tensor_copy(out=h1p[:, :512], in_=h1a_ps[:])
            nc.vector.tensor_copy(out=h1p[:, 512:], in_=h1b_ps[:])
            nc.sync.dma_start(out=h1p_d[:], in_=h1p[:])
    nc.compile()
    return nc


def build_l2():
    NG2 = (50 - NIN1) // 5
    nc = bacc.Bacc("TRN2", target_bir_lowering=False)
    pfull_d = nc.dram_tensor("pfull", [G, NOISE], f32, kind="ExternalInput")
    h1p_d = nc.dram_tensor("h1p", [OPC, D0], f32, kind="ExternalInput")
    meT_d = nc.dram_tensor("meT", [32, BPC], bf16, kind="ExternalInput")
    gid_d = nc.dram_tensor("gid", [BPC], i32, kind="ExternalInput")
    gmewB_d = nc.dram_tensor("gmewB", [32, NOISE], bf16, kind="ExternalInput")
    gmeb_d = nc.dram_tensor("gmeb", [NOISE, 1], f32, kind="ExternalInput")
    w0b_d = nc.dram_tensor("w0b", [P, 50 - NIN1, D0], bf16,
                           kind="ExternalInput")
    b0_d = nc.dram_tensor("b0", [1, D0], f32, kind="ExternalInput")
    w1_d = nc.dram_tensor("w1", [P, 8, D1], bf16, kind="ExternalInput")
    b1_d = nc.dram_tensor("b1", [1, D1], f32, kind="ExternalInput")
    w2_d = nc.dram_tensor("w2", [P, 4, 1], bf16, kind="ExternalInput")
    b2_d = nc.dram_tensor("b2", [1, 1], f32, kind="ExternalInput")
    y_d = nc.dram_tensor("y", [OPC, 1], f32, kind="ExternalOutput")

    with tile.TileContext(nc) as tc:
        with (
            tc.tile_pool(name="const", bufs=1) as cpool,
            tc.tile_pool(name="sbuf", bufs=1) as pool,
            tc.tile_pool(name="w0p", bufs=2) as w0pool,
            tc.tile_pool(name="ps", bufs=1, space="PSUM") as ps,
            tc.tile_pool(name="ps2", bufs=2, space="PSUM") as ps2,
        ):
            ident = cpool.tile([P, P], bf16)
            make_identity(nc, ident[:])
            i64c = cpool.tile([G, 1], i32)
            nc.gpsimd.iota(i64c[:], pattern=[[0, 1]], base=0, channel_multiplier=1)
            ones = cpool.tile([1, OPC], bf16)
            nc.vector.memset(ones[:], 1.0)

            gidr = pool.tile([G, BPC], i32)
            nc.scalar.dma_start(out=gidr[:],
                                in_=gid_d[None, :].to_broadcast([G, BPC]))
            ohgid = pool.tile([G, BPC], bf16)
            nc.vector.tensor_tensor(out=ohgid[:], in0=i64c[:].to_broadcast(
                [G, BPC]), in1=gidr[:], op=AOT.is_equal)

            Pfull = pool.tile([G, NOISE], f32)
            nc.gpsimd.dma_start(out=Pfull[:], in_=pfull_d[:])
            P_sb = pool.tile([G, NOISE], bf16)
            nc.vector.tensor_copy(out=P_sb[:], in_=Pfull[:])
            meT = pool.tile([32, BPC], bf16)
            nc.gpsimd.dma_start(out=meT[:], in_=meT_d[:])
            wB = pool.tile([32, NOISE], bf16)
            nc.scalar.dma_start(out=wB[:], in_=gmewB_d[:])
            gb = cpool.tile([NOISE, 1], f32)
            nc.scalar.dma_start(out=gb[:], in_=gmeb_d[:])
            h1pt = pool.tile([OPC, D0], f32)
            nc.gpsimd.dma_start(out=h1pt[:], in_=h1p_d[:])

            nz_ps = ps.tile([NOISE, BPC], f32, tag="psN")
            for c0 in range(0, BPC, 512):
                cw = min(512, BPC - c0)
                nc.tensor.matmul(out=nz_ps[:, c0:c0 + cw], lhsT=P_sb[:],
                                 rhs=ohgid[:, c0:c0 + cw], start=True, stop=False)
                nc.tensor.matmul(out=nz_ps[:, c0:c0 + cw], lhsT=wB[:],
                                 rhs=meT[:, c0:c0 + cw], start=False, stop=True)
            nzT = pool.tile([NOISE, BPC], bf16)
            nc.scalar.activation(out=nzT[:], in_=nz_ps[:],
                                 func=ACTF.Identity, bias=gb[:, :1], scale=1.0)

            # h1: noise-dependent chunks + bias, then add L1 partial
            b0 = pool.tile([1, D0], f32)
            nc.scalar.dma_start(out=b0[:], in_=b0_d[:])
            b0b = pool.tile([1, D0], bf16)
            nc.vector.tensor_copy(out=b0b[:], in_=b0[:])
            h1a_ps = ps.tile([OPC, 512], f32, tag="psA")
            h1b_ps = ps.tile([OPC, 512], f32, tag="psA2")
            order = [c for c in range(50) if c % 5 != 4] + \
                    [c for c in range(50) if c % 5 == 4]
            first = True
            for cg in range(NG2):
                w0g = w0pool.tile([P, 5, D0], bf16, tag="w0g")
                nc.sync.dma_start(out=w0g[:],
                                  in_=w0b_d[:, 5 * cg:5 * cg + 5, :])
                for ci in range(5):
                    c = order[NIN1 + 5 * cg + ci]
                    p_, fb = c // 5, c % 5
                    if fb < 4:
                        raise AssertionError("L2 expects noise chunks only")
                    lhs = nzT[:, p_::PAC]
                    nc.tensor.matmul(out=h1a_ps[:], lhsT=lhs,
                                     rhs=w0g[:, ci, :512],
                                     start=first, stop=False)
                    nc.tensor.matmul(out=h1b_ps[:], lhsT=lhs,
                                     rhs=w0g[:, ci, 512:],
                                     start=first, stop=False)
                    first = False
            nc.tensor.matmul(out=h1a_ps[:], lhsT=ones[:],
                             rhs=b0b[:, :512], start=False, stop=True)
            nc.tensor.matmul(out=h1b_ps[:], lhsT=ones[:],
                             rhs=b0b[:, 512:], start=False, stop=True)
            # h1pre = psum + L1 partial; lrelu
            h1f = pool.tile([OPC, D0], f32)
            nc.vector.tensor_tensor(out=h1f[:, :512], in0=h1a_ps[:],
                                    in1=h1pt[:, :512], op=AOT.add)
            nc.vector.tensor_tensor(out=h1f[:, 512:], in0=h1b_ps[:],
                                    in1=h1pt[:, 512:], op=AOT.add)
            h1s = pool.tile([OPC, D0], f32)
            nc.vector.tensor_scalar(out=h1s[:], in0=h1f[:], scalar1=0.2,
                                    scalar2=None, op0=AOT.mult)
            h1b = pool.tile([OPC, D0], bf16)
            nc.vector.tensor_tensor(out=h1b[:], in0=h1f[:], in1=h1s[:],
                                    op=AOT.max)

            # transpose h1 -> [128, 8, 125]
            h1T = pool.tile([P, 8, OPC], bf16)
            for b in range(8):
                tp = ps2.tile([P, OPC], bf16, tag="tp")
                nc.tensor.transpose(out=tp[:], in_=h1b[:, 128 * b:128 * b + 128],
                                    identity=ident[:OPC, :OPC])
                nc.vector.tensor_copy(out=h1T[:, b, :], in_=tp[:])

            # h2 = lrelu(h1 @ w1 + b1)
            w1 = pool.tile([P, 8, D1], bf16)
            nc.gpsimd.dma_start(out=w1[:], in_=w1_d[:])
            b1 = pool.tile([1, D1], f32)
            nc.scalar.dma_start(out=b1[:], in_=b1_d[:])
            b1b = pool.tile([1, D1], bf16)
            nc.vector.tensor_copy(out=b1b[:], in_=b1[:])
            h2_ps = ps.tile([OPC, D1], f32, tag="psB")
            for c in range(8):
                nc.tensor.matmul(out=h2_ps[:], lhsT=h1T[:, c, :], rhs=w1[:, c, :],
                                 start=(c == 0), stop=False)
            nc.tensor.matmul(out=h2_ps[:], lhsT=ones[:], rhs=b1b[:],
                             start=False, stop=True)
            h2 = pool.tile([OPC, D1], f32)
            nc.vector.tensor_scalar(out=h2[:], in0=h2_ps[:], scalar1=0.2,
                                    scalar2=None, op0=AOT.mult)
            h2b = pool.tile([OPC, D1], bf16)
            nc.vector.tensor_tensor(out=h2b[:], in0=h2_ps[:], in1=h2[:],
                                    op=AOT.max)

            # transpose h2 -> [128, 4, 125]
            h2T = pool.tile([P, 4, OPC], bf16)
            for b in range(4):
                tp2 = ps2.tile([P, OPC], bf16, tag="tp")
                nc.tensor.transpose(out=tp2[:], in_=h2b[:, 128 * b:128 * b + 128],
                                    identity=ident[:OPC, :OPC])
                nc.vector.tensor_copy(out=h2T[:, b, :], in_=tp2[:])

            # y = h2 @ w2 + b2
            w2 = pool.tile([P, 4, 1], bf16)
            nc.scalar.dma_start(out=w2[:], in_=w2_d[:])
            b2 = pool.tile([1, 1], f32)
            nc.scalar.dma_start(out=b2[:], in_=b2_d[:])
            b2b = pool.tile([1, 1], bf16)
            nc.vector.tensor_copy(out=b2b[:], in_=b2[:])
            y_ps = ps.tile([OPC, 1], f32, tag="psB")
            for c in range(4):
                nc.tensor.matmul(out=y_ps[:], lhsT=h2T[:, c, :], rhs=w2[:, c, :],
                                 start=(c == 0), stop=False)
            nc.tensor.matmul(out=y_ps[:], lhsT=ones[:], rhs=b2b[:],
                             start=False, stop=True)
            y = pool.tile([OPC, 1], f32)
            nc.vector.tensor_copy(out=y[:], in_=y_ps[:])
            nc.sync.dma_start(out=y_d[:], in_=y[:])
    nc.compile()
    return nc


def build_mlp():
    nc = bacc.Bacc("TRN2", target_bir_lowering=False)
    inT_d = nc.dram_tensor("inT", [P, 4, BPC], bf16, kind="ExternalInput")
    amT_d = nc.dram_tensor("amT", [16, BPC], bf16, kind="ExternalInput")
    gid_d = nc.dram_tensor("gid", [BPC], i32, kind="ExternalInput")
    gcnT_d = nc.dram_tensor("gcnT", [P, 16, G], bf16, kind="ExternalInput")
    gmew_d = nc.dram_tensor("gmew", [P, 16, NOISE], bf16, kind="ExternalInput")
    gmewB_d = nc.dram_tensor("gmewB", [32, NOISE], bf16, kind="ExternalInput")
    gmeb_d = nc.dram_tensor("gmeb", [NOISE, 1], f32, kind="ExternalInput")
    metaw_d = nc.dram_tensor("metaw", [16, 32], bf16, kind="ExternalInput")
    metab_d = nc.dram_tensor("metab", [32, 1], f32, kind="ExternalInput")
    w0_d = nc.dram_tensor("w0", [P, 50, D0], bf16, kind="ExternalInput")
    b0_d = nc.dram_tensor("b0", [1, D0], f32, kind="ExternalInput")
    w1_d = nc.dram_tensor("w1", [P, 8, D1], bf16, kind="ExternalInput")
    b1_d = nc.dram_tensor("b1", [1, D1], f32, kind="ExternalInput")
    w2_d = nc.dram_tensor("w2", [P, 4, 1], bf16, kind="ExternalInput")
    b2_d = nc.dram_tensor("b2", [1, 1], f32, kind="ExternalInput")
    y_d = nc.dram_tensor("y", [OPC, 1], f32, kind="ExternalOutput")

    with tile.TileContext(nc) as tc:
        with (
            tc.tile_pool(name="const", bufs=1) as cpool,
            tc.tile_pool(name="sbuf", bufs=2) as pool,
            tc.tile_pool(name="w0p", bufs=2) as w0pool,
            tc.tile_pool(name="ps", bufs=1, space="PSUM") as ps,
            tc.tile_pool(name="ps2", bufs=2, space="PSUM") as ps2,
        ):
            ident = cpool.tile([P, P], bf16)
            make_identity(nc, ident[:])
            i64c = cpool.tile([G, 1], i32)
            nc.gpsimd.iota(i64c[:], pattern=[[0, 1]], base=0, channel_multiplier=1)
            ones = cpool.tile([1, OPC], bf16)
            nc.vector.memset(ones[:], 1.0)

            # one-hot of graph ids (transposed): [64, 1250]
            gidr = pool.tile([G, BPC], i32)
            nc.sync.dma_start(out=gidr[:],
                              in_=gid_d[None, :].to_broadcast([G, BPC]))
            ohgid = pool.tile([G, BPC], bf16)
            nc.vector.tensor_tensor(out=ohgid[:], in0=i64c[:].to_broadcast(
                [G, BPC]), in1=gidr[:], op=AOT.is_equal)

            # P = gcn_out @ gme_w[:2000]  -> [64, 128]
            gcnT = pool.tile([P, 16, G], bf16)
            nc.sync.dma_start(out=gcnT[:], in_=gcnT_d[:])
            gmw = pool.tile([P, 16, NOISE], bf16)
            nc.sync.dma_start(out=gmw[:], in_=gmew_d[:])
            P_ps = ps.tile([G, NOISE], f32, tag="psB")
            for c in range(16):
                nc.tensor.matmul(out=P_ps[:], lhsT=gcnT[:, c, :], rhs=gmw[:, c, :],
                                 start=(c == 0), stop=(c == 15))
            P_sb = pool.tile([G, NOISE], bf16)
            nc.vector.tensor_copy(out=P_sb[:], in_=P_ps[:])

            # meta_emb^T = relu(meta_w^T @ all_meta^T + b) -> [32, 1250] bf16
            amT = pool.tile([16, BPC], bf16)
            nc.sync.dma_start(out=amT[:], in_=amT_d[:])
            mw = pool.tile([16, 32], bf16)
            nc.sync.dma_start(out=mw[:], in_=metaw_d[:])
            mb = cpool.tile([32, 1], f32)
            nc.sync.dma_start(out=mb[:], in_=metab_d[:])
            me_ps = ps.tile([32, BPC], f32, tag="psA")
            for c0 in range(0, BPC, 512):
                cw = min(512, BPC - c0)
                nc.tensor.matmul(out=me_ps[:, c0:c0 + cw], lhsT=mw[:],
                                 rhs=amT[:, c0:c0 + cw], start=True, stop=True)
            meT = pool.tile([32, BPC], bf16)
            nc.scalar.activation(out=meT[:], in_=me_ps[:],
                                 func=ACTF.Relu, bias=mb[:, :1], scale=1.0)

            # noise^T = P^T gathered + wB^T @ meta_emb^T + gme_b -> [128, 1250]
            wB = pool.tile([32, NOISE], bf16)
            nc.sync.dma_start(out=wB[:], in_=gmewB_d[:])
            gb = cpool.tile([NOISE, 1], f32)
            nc.sync.dma_start(out=gb[:], in_=gmeb_d[:])
            nz_ps = ps.tile([NOISE, BPC], f32, tag="psA")
            for c0 in range(0, BPC, 512):
                cw = min(512, BPC - c0)
                nc.tensor.matmul(out=nz_ps[:, c0:c0 + cw], lhsT=P_sb[:],
                                 rhs=ohgid[:, c0:c0 + cw], start=True, stop=False)
                nc.tensor.matmul(out=nz_ps[:, c0:c0 + cw], lhsT=wB[:],
                                 rhs=meT[:, c0:c0 + cw], start=False, stop=True)
            nzT = pool.tile([NOISE, BPC], bf16)
            nc.scalar.activation(out=nzT[:], in_=nz_ps[:],
                                 func=ACTF.Identity, bias=gb[:, :1], scale=1.0)

            # input^T
            inT = pool.tile([P, 4, BPC], bf16)
            nc.sync.dma_start(out=inT[:], in_=inT_d[:])

            # h1 = lrelu(h @ w0 + b0): psum [125, 1024]
            b0 = pool.tile([1, D0], f32)
            nc.sync.dma_start(out=b0[:], in_=b0_d[:])
            h1a_ps = ps.tile([OPC, 512], f32, tag="psA")
            h1b_ps = ps.tile([OPC, 512], f32, tag="psA2")
            b0b = pool.tile([1, D0], bf16)
            nc.vector.tensor_copy(out=b0b[:], in_=b0[:])
            # w0 chunks reordered (host side): input-only chunks (fb<4) first,
            # noise-dependent chunks (fb==4) last, so h1 matmuls start as soon
            # as the first weight group lands.
            order = [c for c in range(50) if c % 5 != 4] + \
                    [c for c in range(50) if c % 5 == 4]
            first = True
            for cg in range(5):
                w0g = w0pool.tile([P, 10, D0], bf16, tag="w0g")
                nc.sync.dma_start(out=w0g[:],
                                  in_=w0_d[:, 10 * cg:10 * cg + 10, :])
                for ci in range(10):
                    c = order[10 * cg + ci]
                    p_, fb = c // 5, c % 5
                    if fb < 4:
                        lhs = inT[:, fb, p_::PAC]
                    else:
                        lhs = nzT[:, p_::PAC]
                    nc.tensor.matmul(out=h1a_ps[:], lhsT=lhs,
                                     rhs=w0g[:, ci, :512],
                                     start=first, stop=False)
                    nc.tensor.matmul(out=h1b_ps[:], lhsT=lhs,
                                     rhs=w0g[:, ci, 512:],
                                     start=first, stop=False)
                    first = False
            nc.tensor.matmul(out=h1a_ps[:], lhsT=ones[:],
                             rhs=b0b[:, :512], start=False, stop=True)
            nc.tensor.matmul(out=h1b_ps[:], lhsT=ones[:],
                             rhs=b0b[:, 512:], start=False, stop=True)
            h1 = pool.tile([OPC, D0], f32)
            h1b = pool.tile([OPC, D0], bf16)
            nc.vector.tensor_scalar(out=h1[:, :512], in0=h1a_ps[:], scalar1=0.2,
                                    scalar2=None, op0=AOT.mult)
            nc.vector.tensor_scalar(out=h1[:, 512:], in0=h1b_ps[:], scalar1=0.2,
                                    scalar2=None, op0=AOT.mult)
            nc.vector.tensor_tensor(out=h1b[:, :512], in0=h1a_ps[:],
                                    in1=h1[:, :512], op=AOT.max)
            nc.vector.tensor_tensor(out=h1b[:, 512:], in0=h1b_ps[:],
                                    in1=h1[:, 512:], op=AOT.max)

            # transpose h1 -> [128, 8, 125]
            h1T = pool.tile([P, 8, OPC], bf16)
            for b in range(8):
                tp = ps2.tile([P, OPC], bf16, tag="tp")
                nc.tensor.transpose(out=tp[:], in_=h1b[:, 128 * b:128 * b + 128],
                                    identity=ident[:OPC, :OPC])
                nc.vector.tensor_copy(out=h1T[:, b, :], in_=tp[:])

            # h2 = lrelu(h1 @ w1 + b1): [125, 512]
            w1 = pool.tile([P, 8, D1], bf16)
            nc.sync.dma_start(out=w1[:], in_=w1_d[:])
            b1 = pool.tile([1, D1], f32)
            nc.sync.dma_start(out=b1[:], in_=b1_d[:])
            b1b = pool.tile([1, D1], bf16)
            nc.vector.tensor_copy(out=b1b[:], in_=b1[:])
            h2_ps = ps.tile([OPC, D1], f32, tag="psB")
            for c in range(8):
                nc.tensor.matmul(out=h2_ps[:], lhsT=h1T[:, c, :], rhs=w1[:, c, :],
                                 start=(c == 0), stop=False)
            nc.tensor.matmul(out=h2_ps[:], lhsT=ones[:], rhs=b1b[:],
                             start=False, stop=True)
            h2 = pool.tile([OPC, D1], f32)
            nc.vector.tensor_scalar(out=h2[:], in0=h2_ps[:], scalar1=0.2,
                                    scalar2=None, op0=AOT.mult)
            h2b = pool.tile([OPC, D1], bf16)
            nc.vector.tensor_tensor(out=h2b[:], in0=h2_ps[:], in1=h2[:],
                                    op=AOT.max)

            # transpose h2 -> [128, 4, 125]
            h2T = pool.tile([P, 4, OPC], bf16)
            for b in range(4):
                tp2 = ps2.tile([P, OPC], bf16, tag="tp")
                nc.tensor.transpose(out=tp2[:], in_=h2b[:, 128 * b:128 * b + 128],
                                    identity=ident[:OPC, :OPC])
                nc.vector.tensor_copy(out=h2T[:, b, :], in_=tp2[:])

            # y = h2 @ w2 + b2: [125, 1]
            w2 = pool.tile([P, 4, 1], bf16)
            nc.sync.dma_start(out=w2[:], in_=w2_d[:])
            b2 = pool.tile([1, 1], f32)
            nc.sync.dma_start(out=b2[:], in_=b2_d[:])
            b2b = pool.tile([1, 1], bf16)
            nc.vector.tensor_copy(out=b2b[:], in_=b2[:])
            y_ps = ps.tile([OPC, 1], f32, tag="psB")
            for c in range(4):
                nc.tensor.matmul(out=y_ps[:], lhsT=h2T[:, c, :], rhs=w2[:, c, :],
                                 start=(c == 0), stop=False)
            nc.tensor.matmul(out=y_ps[:], lhsT=ones[:], rhs=b2b[:],
                             start=False, stop=True)
            y = pool.tile([OPC, 1], f32)
            nc.vector.tensor_copy(out=y[:], in_=y_ps[:])
            nc.sync.dma_start(out=y_d[:], in_=y[:])
    nc.compile()
    return nc


_CACHE = {}
_MODE = os.environ.get("KMODE", "split")


def _programs(kmax):
    if "gcn" not in _CACHE:
        _install_ntff_hook()
        _CACHE["gcn"] = build_gcn(kmax)
        _CACHE["mlp"] = build_mlp()
    return _CACHE["gcn"], _CACHE["mlp"]


def _program_fused(kmax):
    if "fused" not in _CACHE:
        _install_ntff_hook()
        _CACHE["fused"] = build_fused(kmax)
    return _CACHE["fused"]


def _programs_split(kmax):
    if "l1" not in _CACHE:
        _install_ntff_hook()
        _CACHE["l1"] = build_l1(kmax)
        _CACHE["l2"] = build_l2()
    return _CACHE["l1"], _CACHE["l2"]


def _bf(x):
    import ml_dtypes
    return np.asarray(x, dtype=ml_dtypes.bfloat16)


def _csr_prep(graphs_x, edge_index):
    """Integer-only CSR layout prep + pure gathers of input data.

    Returns per-graph slot tensors (x[src] per slot, deg+1 per slot as
    bf16-encoded ints), per-node deg+1 and x. No float arithmetic here;
    everything numeric the device consumes is either gathered input data
    or integer counts of index values.
    """
    f = np.float32
    ei = np.asarray(edge_index, np.int64)
    src, dst = ei[:, 0, :], ei[:, 1, :]            # [G, E]
    xpad = np.zeros((G, NPAD), f)
    xpad[:, :N] = np.asarray(graphs_x, f).reshape(G, N)

    # deg+1 per node (integer bincount of dst indices)
    deg1 = np.ones((G, NPAD), np.int32)
    for g in range(G):
        deg1[g, :N] += np.bincount(dst[g], minlength=N)[:N]
    kmax = int(deg1.max())                          # includes the +1
    kmax = max(16, -(-kmax // 8) * 8)

    # slot position of each edge: k-th edge into its dst node
    order = np.argsort(dst, axis=1, kind='stable')
    dsts = np.take_along_axis(dst, order, axis=1)
    srcs = np.take_along_axis(src, order, axis=1)
    starts = np.zeros((G, NPAD), np.int64)
    for g in range(G):
        cs = np.cumsum(np.bincount(dsts[g], minlength=NPAD))
        starts[g, 1:] = cs[:-1]
    k = np.arange(E)[None, :] - np.take_along_axis(starts, dsts, axis=1)
    flat = dsts * kmax + k                          # [G, E] slot index

    xi = np.zeros((G, NPAD * kmax), f)
    dsl = np.ones((G, NPAD * kmax), np.int32)
    gi = np.arange(G)[:, None]
    xi[gi, flat] = xpad[gi, srcs]
    dsl[gi, flat] = deg1[gi, srcs]
    xi = _bf(xi).reshape(G, P, WN, kmax)
    dsl = _bf(dsl).reshape(G, P, WN, kmax)
    # dnx[core]: [P, {deg+1, x}, GPC, WN] f32
    dn = deg1.astype(f).reshape(NCORES, GPC, P, WN)
    xn = xpad.reshape(NCORES, GPC, P, WN)
    dnx = np.ascontiguousarray(
        np.stack([dn, xn], axis=1).transpose(0, 3, 1, 2, 4))
    return kmax, xi, dsl, dnx


def _mlp_host_prep(input_, graph_ids, chain, metadata, gme_w, gme_b,
                   meta_w, meta_b, seq_w0, seq_b0, seq_w1, seq_b1,
                   seq_w2, seq_b2):
    f = np.float32
    inT_full = np.ascontiguousarray(np.asarray(input_, f).T)
    amT_full = np.ascontiguousarray(
        np.concatenate([np.asarray(chain, f), np.asarray(metadata, f)], axis=1).T)
    w0b = _bf(seq_w0)
    _order = [c for c in range(50) if c % 5 != 4] + \
             [c for c in range(50) if c % 5 == 4]
    w0r = np.ascontiguousarray(
        w0b.reshape(50, P, D0)[_order].transpose(1, 0, 2))
    w1b = np.ascontiguousarray(
        _bf(seq_w1).reshape(8, P, D1).transpose(1, 0, 2))
    w2b = np.ascontiguousarray(
        _bf(seq_w2).reshape(4, P, 1).transpose(1, 0, 2))
    gmewBb = _bf(np.asarray(gme_w, f)[N:, :])
    metawb = _bf(meta_w)
    in2 = []
    for c in range(NCORES):
        sl = slice(BPC * c, BPC * c + BPC)
        in2.append({
            "inT": np.ascontiguousarray(
                _bf(inT_full[:, sl]).reshape(4, P, BPC).transpose(1, 0, 2)),
            "amT": _bf(amT_full[:, sl]),
            "gid": np.ascontiguousarray(np.asarray(graph_ids, np.int32)[sl]),
            "gmewB": gmewBb,
            "gmeb": np.asarray(gme_b, f).reshape(NOISE, 1),
            "metaw": metawb,
            "metab": np.asarray(meta_b, f).reshape(32, 1),
            "w0": w0r,
            "b0": np.asarray(seq_b0, f).reshape(1, D0),
            "w1": w1b, "b1": np.asarray(seq_b1, f).reshape(1, D1),
            "w2": w2b, "b2": np.asarray(seq_b2, f).reshape(1, 1),
        })
    return in2


def kernel(input_, graphs_x, edge_index, graph_ids, chain, metadata,
           gcn_w, gcn_b, meta_w, meta_b, gme_w, gme_b,
           seq_w0, seq_b0, seq_w1, seq_b1, seq_w2, seq_b2,
           _trace=False):
    f = np.float32
    kmax, xi, dsl, dnx = _csr_prep(graphs_x, edge_index)
    wb = np.empty((P, 2), f)
    wb[:, 0] = np.float32(gcn_w.reshape(-1)[0])
    wb[:, 1] = np.float32(gcn_b.reshape(-1)[0])
    gmew_pad = np.zeros((NPAD, NOISE), f)
    gmew_pad[:N, :] = np.asarray(gme_w, f)[:N, :]
    in2 = _mlp_host_prep(input_, graph_ids, chain, metadata, gme_w, gme_b,
                         meta_w, meta_b, seq_w0, seq_b0, seq_w1, seq_b1,
                         seq_w2, seq_b2)

    if _MODE == "split":
        nc1, nc2 = _programs_split(kmax)
        gmwvb = np.ascontiguousarray(_bf(gmew_pad).reshape(P, WN, NOISE))
        in1 = []
        for c in range(NCORES):
            sl = slice(GPC * c, GPC * c + GPC)
            m2 = in2[c]
            in1.append({
                "xi": np.ascontiguousarray(xi[sl]),
                "dsl": np.ascontiguousarray(dsl[sl]),
                "dnx": dnx[c],
                "wb": wb,
                "gmwv": gmwvb,
                "inT": m2["inT"],
                "w0a": np.ascontiguousarray(m2["w0"][:, :NIN1, :]),
                "amT": m2["amT"],
                "metaw": m2["metaw"],
                "metab": m2["metab"],
            })
        r1 = bass_utils.run_bass_kernel_spmd(nc1, in1,
                                             core_ids=list(range(NCORES)),
                                             trace=_trace)
        pfull = np.ascontiguousarray(np.concatenate(
            [np.asarray(r1.results[c]["pp"], f) for c in range(NCORES)],
            axis=0))
        inL2 = []
        for c in range(NCORES):
            m2 = in2[c]
            inL2.append({
                "pfull": pfull,
                "h1p": np.ascontiguousarray(np.asarray(r1.results[c]["h1p"], f)),
                "meT": np.ascontiguousarray(np.asarray(r1.results[c]["meTo"])),
                "gid": m2["gid"],
                "gmewB": m2["gmewB"],
                "gmeb": m2["gmeb"],
                "w0b": np.ascontiguousarray(m2["w0"][:, NIN1:, :]),
                "b0": m2["b0"],
                "w1": m2["w1"], "b1": m2["b1"],
                "w2": m2["w2"], "b2": m2["b2"],
            })
        r2 = bass_utils.run_bass_kernel_spmd(nc2, inL2,
                                             core_ids=list(range(NCORES)),
                                             trace=_trace)
        y = np.concatenate([np.asarray(r2.results[c]["y"], f)
                            for c in range(NCORES)], axis=0)
        kernel.last_exec_ns = ((r1.exec_time_ns or 0), (r2.exec_time_ns or 0))
        return y

    if _MODE == "fused":
        ncf = _program_fused(kmax)
        gmwvb = np.ascontiguousarray(_bf(gmew_pad).reshape(P, WN, NOISE))
        inf = []
        for c in range(NCORES):
            sl = slice(GPC * c, GPC * c + GPC)
            m = dict(in2[c])
            m.update({
                "xi": np.ascontiguousarray(xi[sl]),
                "dsl": np.ascontiguousarray(dsl[sl]),
                "dnx": dnx[c],
                "wb": wb,
                "gmwv": gmwvb,
            })
            inf.append(m)
        r = bass_utils.run_bass_kernel_spmd(ncf, inf,
                                            core_ids=list(range(NCORES)),
                                            trace=_trace)
        y = np.concatenate([np.asarray(r.results[c]["y"], f)
                            for c in range(NCORES)], axis=0)
        kernel.last_exec_ns = ((r.exec_time_ns or 0), 0)
        return y

    nc1, nc2 = _programs(kmax)
    in1 = []
    for c in range(NCORES):
        sl = slice(GPC * c, GPC * c + GPC)
        in1.append({
            "xi": np.ascontiguousarray(xi[sl]),
            "dsl": np.ascontiguousarray(dsl[sl]),
            "dnx": dnx[c],
            "wb": wb,
        })
    r1 = bass_utils.run_bass_kernel_spmd(nc1, in1, core_ids=list(range(NCORES)),
                                         trace=_trace)
    # go: [P, GPC, WN] f32 per core, node v = 16*p + w -> gcnT [NPAD, G]
    gcnT = np.concatenate(
        [np.asarray(r1.results[c]["go"], f).transpose(1, 0, 2).reshape(GPC, NPAD)
         for c in range(NCORES)], axis=0).T

    gcnTb = np.ascontiguousarray(
        _bf(gcnT).reshape(16, P, G).transpose(1, 0, 2))
    gmewb = np.ascontiguousarray(
        _bf(gmew_pad).reshape(16, P, NOISE).transpose(1, 0, 2))
    for c in range(NCORES):
        in2[c]["gcnT"] = gcnTb
        in2[c]["gmew"] = gmewb
    r2 = bass_utils.run_bass_kernel_spmd(nc2, in2, core_ids=list(range(NCORES)),
                                         trace=_trace)
    y = np.concatenate([np.asarray(r2.results[c]["y"], f)
                        for c in range(NCORES)], axis=0)
    kernel.last_exec_ns = ((r1.exec_time_ns or 0), (r2.exec_time_ns or 0))
    return y
